# revision 1
# baseline (speedup 1.0000x reference)
"""Trainium2 Bass kernel for nn_DataManifolds_18915035972342 (gnn_message_passing).

Self-contained: builds an 8-core SPMD Bass/Tile program on first call,
shards the 1000 runs across 8 NeuronCores (125 runs each, batches of 25),
runs the full per-run pipeline on-device, and gathers the per-run accuracy.

Per run (n=100 nodes = 75 queries + 25 support, permuted queries-first so all
engine ops start at partition 0; d=640):
  A1 = exp(-lam*sqdist(X,X));  Xc = ((I + D^-.5 A1 D^-.5)/2)^4 @ X   (elemwise ^4)
  A2 = exp(-lam*sqdist(Xc,Xc)) with diagonal killed; W = row-top-20(A2),
       symmetrically normalized;  Y = (I - alpha*W)^-T  (Newton, bf16 matmuls)
  5 epochs of { dist->exp->Sinkhorn(2) -> Y@Z -> relu -> clamped Sinkhorn(2)
                -> proto update } ; final argmax accuracy per run.

Implementation highlights:
- sqdist via one PE accumulation chain per run: 5 K-tiles of X^T (bf16) plus
  an augmented K=2 tile carrying (-qn/2_i - qn/2_j), plus a K=100 identity
  tile adding a -25 diagonal penalty (kills the diagonal after exp).
- All proto math in z-space (proto = Xc^T z): epochs touch only [100,5]
  mms against the stored dist-gram, never the [*,640] features.
- Top-20 threshold via DVE max8/match_replace (3 passes -> 20th largest).
- Degree^-1/2 via DVE fast-inverse-sqrt bit trick (keeps ACT on a single
  exp/copy/relu table set -- no ACT_TABLE_LOAD churn).
- Phase-1 processed in groups of 5 runs sharing [100,500] tiles/PSUM banks;
  phase-2 (Sinkhorn) packs 25 runs per instruction ([100,125] tiles).
"""

import numpy as np
from contextlib import ExitStack

import concourse.bass as bass
import concourse.tile as tile
from concourse import bacc, mybir

alu = mybir.AluOpType
actf = mybir.ActivationFunctionType
axl = mybir.AxisListType
BF = mybir.dt.bfloat16
F32 = mybir.dt.float32

N, NS, QS, WAYS, DIM = 100, 25, 75, 5, 640
LAM, ALPHA, UR, EPOCHS = 10.0, 0.7, 0.6, 5
SINK_ITERS = 2
NEWTON_ITERS = 3
DIAG_PEN = 50.0
G = 5                       # phase-1 group size
LN_ALPHA = float(np.log(ALPHA))
USE_RECIP_APPROX = True


def node_perm():
    return np.concatenate([np.arange(NS, N), np.arange(NS)])


def host_consts(B):
    import ml_dtypes
    bf = ml_dtypes.bfloat16
    eye = np.eye(128, dtype=np.float32)
    ys = np.repeat(np.arange(WAYS), NS // WAYS)
    ms = np.zeros((N, WAYS), np.float32)
    ms[QS + np.arange(NS), ys] = 1.0 / (NS // WAYS)
    oh = np.zeros((NS, WAYS), np.float32)
    oh[np.arange(NS), ys] = 1.0
    return {
        "ident": eye.astype(bf),
        "eyeN": np.eye(N, dtype=np.float32),
        "halfI": (0.5 * np.eye(N)).astype(np.float32),
        "twoI": (2.0 * np.eye(N)).astype(np.float32),
        "nid": (-0.5 * DIAG_PEN * np.eye(N)).astype(bf),
        "msp": np.tile(ms, (1, B)).astype(bf),
        "ohp": np.tile(oh, (1, B)).astype(bf),
        "ones_col": np.ones((128, 1), np.float32).astype(bf),
        "ones_row": np.ones((1, 128), np.float32).astype(bf),
        "ones_colf": np.ones((128, 1), np.float32),
        "ones_rowf": np.ones((1, 128), np.float32),
        "qmask": np.concatenate([np.ones((QS, 1), np.float32),
                                 np.zeros((NS, 1), np.float32)]),
    }


def host_inputs(xs, xq, yq, NB, B):
    import ml_dtypes
    bf = ml_dtypes.bfloat16
    R = xs.shape[0]
    assert NB * B == R
    perm = node_perm()
    feat = np.concatenate([xs, xq], axis=1)[:, perm, :]
    x0 = np.ascontiguousarray(feat).astype(bf)
    xt = np.ascontiguousarray(np.swapaxes(x0, 1, 2))
    # gram1 aug rows from the bf16 values (consistent with PE products)
    x0f = x0.astype(np.float32)
    qn_h = -0.5 * (x0f * x0f).sum(2)                     # [R,100]
    aug1 = np.zeros((R, 4, N), np.float32)
    aug1[:, 0] = qn_h
    aug1[:, 1] = 1.0
    aug1[:, 2] = 1.0
    aug1[:, 3] = qn_h
    yq1 = (yq[:, :, None] == np.arange(WAYS)[None, None, :]).astype(np.float32)
    yqp = np.ascontiguousarray(
        yq1.reshape(NB, B, QS, WAYS).transpose(0, 2, 1, 3)
    ).reshape(NB, QS, B * WAYS).astype(bf)
    out = {"x0": x0, "xt": xt, "aug1": aug1.astype(bf), "yqp": yqp}
    out.update(host_consts(B))
    return out


def declare_dram(nc, R, NB, B):
    BW = B * WAYS
    mk = lambda n, s, dt, k="ExternalInput": nc.dram_tensor(n, s, dt, kind=k).ap()
    return {
        "x0": mk("x0", [R, N, DIM], BF),
        "xt": mk("xt", [R, DIM, N], BF),
        "aug1": mk("aug1", [R, 4, N], BF),
        "yqp": mk("yqp", [NB, QS, BW], BF),
        "ident": mk("ident", [128, 128], BF),
        "eyeN": mk("eyeN", [N, N], F32),
        "halfI": mk("halfI", [N, N], F32),
        "twoI": mk("twoI", [N, N], F32),
        "nid": mk("nid", [N, N], BF),
        "msp": mk("msp", [N, BW], BF),
        "ohp": mk("ohp", [NS, BW], BF),
        "ones_col": mk("ones_col", [128, 1], BF),
        "ones_row": mk("ones_row", [1, 128], BF),
        "ones_colf": mk("ones_colf", [128, 1], F32),
        "ones_rowf": mk("ones_rowf", [1, 128], F32),
        "qmask": mk("qmask", [N, 1], F32),
        "acc": mk("acc", [R], F32, "ExternalOutput"),
    }


def bc3(ap, g):
    """[100,100] const -> broadcast view [100, g, 100] (step-0 middle dim)."""
    return ap.unsqueeze(1).broadcast_to((ap.shape[0], g, ap.shape[1]))


class Kernel:
    def __init__(self, tc, ctx, d, B, debug=()):
        self.tc, self.ctx, self.d, self.B = tc, ctx, d, B
        self.BW = B * WAYS
        self.nc = tc.nc
        self.debug = set(debug)
        self.dbg_tensors = {}
        big_b = B > 25        # single-batch mode: phase-2 tiles are 2 banks
        p = lambda name, bufs, **kw: ctx.enter_context(
            tc.tile_pool(name=name, bufs=bufs, **kw))
        self.consts = p("consts", 1)
        self.xin = p("xin", 2 if big_b else 3)
        self.gwork = p("gwork", 2 if big_b else 3)
        self.small = p("small", 3 if big_b else 4)
        self.store = p("store", 1 if big_b else 2)
        self.otp = p("otp", 2 if big_b else 3)
        self.psum_g = p("psum_g", 2 if big_b else 3, space="PSUM")
        self.psum_s = p("psum_s", 2, space="PSUM")
        self.psum_2 = p("psum_2", 2 if big_b else 3, space="PSUM")
        self._load_consts()

    def _load_consts(self):
        nc, d = self.nc, self.d
        P = self.consts
        for nm, shape, dt in [
            ("ident", [128, 128], BF), ("eyeN", [N, N], F32),
            ("halfI", [N, N], F32), ("twoI", [N, N], F32),
            ("nid", [N, N], BF),
            ("msp", [N, self.BW], BF),
            ("ones_col", [128, 1], BF), ("ones_row", [1, 128], BF),
            ("ones_colf", [128, 1], F32), ("ones_rowf", [1, 128], F32),
            ("qmask", [N, 1], F32),
        ]:
            t = P.tile(shape, dt, tag=nm)
            nc.sync.dma_start(t[:], d[nm][:])
            setattr(self, nm, t)
        self.rsqmagic = P.tile([128, 8], mybir.dt.uint32, tag="rsqmagic")
        nc.vector.memset(self.rsqmagic[:], 0x5F3759DF)
        self.ohp = P.tile([N, self.BW], BF, tag="ohp")
        nc.vector.memset(self.ohp[0:QS, :], 0.0)
        nc.sync.dma_start(self.ohp[QS:N, :], d["ohp"][:])

    def tap(self, name, ap, r):
        if name not in self.debug:
            return
        nc = self.nc
        key = f"dbg_{name}_{r}"
        t = nc.dram_tensor(key, list(ap.shape), ap.dtype, kind="ExternalOutput").ap()
        self.dbg_tensors[key] = t
        if ap.space == bass.MemorySpace.PSUM:
            s = self.gwork.tile(list(ap.shape), ap.dtype, tag="dbgcpy")
            nc.vector.tensor_copy(s[:], ap)
            ap = s[:]
        nc.sync.dma_start(t, ap)

    def recip(self, out, in_):
        if USE_RECIP_APPROX:
            self.nc.vector.reciprocal_approx_fast(out, in_)
        else:
            self.nc.vector.reciprocal(out, in_)

    # ---------------------------------------------------------------- phase 1
    def degree_rows(self, rs_g, want_scaled):
        """rs_g [100,G] f32 rowsums -> per-run [1,100] bf16 rows of rs^-0.5
        (and alpha*rs^-0.5 if want_scaled). DVE-only fast inverse sqrt
        (bit trick + 2 Newton steps, ~5e-6 rel err) — keeps the ACT table
        set at {exp, copy, relu} for the whole program."""
        nc = self.nc
        U32 = mybir.dt.uint32
        iv = self.small.tile([N, G], U32, tag="rsq_i")
        nc.vector.tensor_scalar(iv[:], rs_g[:].bitcast(U32), 1, None,
                                alu.logical_shift_right)
        nc.vector.tensor_tensor(iv[:], self.rsqmagic[:N, :G], iv[:],
                                alu.subtract)
        yv = self.small.tile([N, G], F32, tag="rsq_y")
        tv = self.small.tile([N, G], F32, tag="rsq_t")
        nc.vector.tensor_copy(yv[:], iv[:].bitcast(F32))
        for _ in range(2):
            nc.vector.tensor_tensor(tv[:], yv[:], yv[:], alu.mult)
            nc.vector.tensor_tensor(tv[:], tv[:], rs_g[:], alu.mult)
            nc.vector.tensor_scalar(tv[:], tv[:], -0.5, 1.5, alu.mult, alu.add)
            nc.vector.tensor_tensor(yv[:], yv[:], tv[:], alu.mult)
        dm_p = self.small.tile([N, G], BF, tag="dm_p")
        nc.vector.tensor_copy(dm_p[:], yv[:])
        dm_s = None
        if want_scaled:
            dm_s = self.small.tile([N, G], BF, tag="dm_s")
            nc.vector.tensor_scalar(dm_s[:], yv[:], ALPHA, None, alu.mult)
        ppb = self.psum_s.tile([1, G * N], BF, tag="ps")
        for i in range(G):
            nc.tensor.transpose(ppb[:, i * N:(i + 1) * N], dm_p[:, i:i + 1],
                                self.ident[:N, :N])
        rpb = self.small.tile([1, G * N], BF, tag="rpb")
        nc.scalar.activation(rpb[:], ppb[:], actf.Copy)
        rows_p = [rpb[:, i * N:(i + 1) * N] for i in range(G)]
        rows_s = []
        if want_scaled:
            spb = self.psum_s.tile([1, G * N], BF, tag="ps")
            for i in range(G):
                nc.tensor.transpose(spb[:, i * N:(i + 1) * N], dm_s[:, i:i + 1],
                                    self.ident[:N, :N])
            rsb = self.small.tile([1, G * N], BF, tag="rsb")
            nc.scalar.activation(rsb[:], spb[:], actf.Copy)
            rows_s = [rsb[:, i * N:(i + 1) * N] for i in range(G)]
        return rows_p, rows_s

    def phase1_group(self, r0, m2a_sl, y_sl):
        """Runs r0..r0+G-1 -> M2a' and Y written into batch-store slices
        (slices are [100, G*100])."""
        nc, d = self.nc, self.d
        GW = G * N
        # ---- loads
        x0 = self.xin.tile([N, G * DIM], BF, tag="x0")
        xts = []
        for k in range(5):
            xtk = self.xin.tile([128, GW], BF, tag=f"xt{k}")
            xts.append(xtk)
        augl = self.xin.tile([2, GW], BF, tag="augl")
        augr = self.xin.tile([2, GW], BF, tag="augr")
        for i in range(G):
            r = r0 + i
            nc.sync.dma_start(x0[:, i * DIM:(i + 1) * DIM], d["x0"][r])
            for k in range(5):
                nc.sync.dma_start(xts[k][:, i * N:(i + 1) * N],
                                  d["xt"][r, 128 * k:128 * (k + 1), :])
            nc.sync.dma_start(augl[:, i * N:(i + 1) * N], d["aug1"][r, 0:2])
            nc.sync.dma_start(augr[:, i * N:(i + 1) * N], d["aug1"][r, 2:4])
        # ---- gram1 (halved): m1' = qn/2_i + qn/2_j - x_i.x_j
        m1 = self.psum_g.tile([N, GW], F32, tag="pg")
        for i in range(G):
            sl = slice(i * N, (i + 1) * N)
            for k in range(5):
                nc.tensor.matmul(m1[:, sl], xts[k][:, sl], xts[k][:, sl],
                                 start=(k == 0), stop=False)
            nc.tensor.matmul(m1[:, sl], augl[:, sl], augr[:, sl],
                             start=False, stop=True)
        a1 = self.gwork.tile([N, GW], F32, tag="a1")
        nc.scalar.activation(a1[:], m1[:], actf.Exp, scale=2.0 * LAM)
        self.tap("a1", a1[:], r0)
        # ---- degree + G^4
        rs1 = self.small.tile([N, G], F32, tag="rs1")
        nc.vector.tensor_reduce(rs1[:], a1[:].rearrange("p (g n) -> p g n", n=N),
                                axl.X, alu.add)
        rows_p, _ = self.degree_rows(rs1, want_scaled=False)
        op = self.psum_g.tile([N, GW], F32, tag="pg")
        for i in range(G):
            nc.tensor.matmul(op[:, i * N:(i + 1) * N], rows_p[i], rows_p[i])
        gw = self.gwork.tile([N, GW], F32, tag="gw")
        nc.vector.tensor_tensor(gw[:], op[:], a1[:], alu.mult)
        gh = self.gwork.tile([N, GW], F32, tag="gh")
        nc.vector.scalar_tensor_tensor(
            gh[:].rearrange("p (g n) -> p g n", n=N),
            gw[:].rearrange("p (g n) -> p g n", n=N), 0.5,
            bc3(self.halfI[:], G), alu.mult, alu.add)
        nc.vector.tensor_tensor(gw[:], gh[:], gh[:], alu.mult)
        g4 = self.gwork.tile([N, GW], BF, tag="g4")
        nc.vector.tensor_tensor(g4[:], gw[:], gw[:], alu.mult)
        self.tap("g4", g4[:], r0)
        # ---- conv Xc = G4 @ X0 (two 320-wide PSUM tiles per run)
        xc = self.gwork.tile([N, G * DIM], BF, tag="xc")
        for i in range(G):
            for h in range(2):
                cp = self.psum_g.tile([N, 320], F32, tag="pg")
                nc.tensor.matmul(cp[:], g4[:, i * N:(i + 1) * N],
                                 x0[:, i * DIM + h * 320:i * DIM + (h + 1) * 320])
                nc.scalar.activation(xc[:, i * DIM + h * 320:i * DIM + (h + 1) * 320],
                                     cp[:], actf.Copy)
        self.tap("xc", xc[:, 0:DIM], r0)
        # ---- qn2 (halved) + aug2 rows via transpose
        qsq = self.xin.tile([N, G * DIM], BF, tag="qsq")
        qn2 = self.small.tile([N, G], F32, tag="qn2")
        for i in range(G):
            nc.vector.scalar_tensor_tensor(
                qsq[:, i * DIM:(i + 1) * DIM], xc[:, i * DIM:(i + 1) * DIM],
                -0.5, xc[:, i * DIM:(i + 1) * DIM], alu.mult, alu.mult,
                accum_out=qn2[:, i:i + 1])
        pre = self.small.tile([N, 2 * G], BF, tag="pre")
        for i in range(G):
            nc.vector.tensor_copy(pre[:, 2 * i:2 * i + 1], qn2[:, i:i + 1])
            nc.vector.tensor_copy(pre[:, 2 * i + 1:2 * i + 2], self.ones_col[:N, :])
        lpb = self.psum_s.tile([2, G * N], BF, tag="ps")
        for i in range(G):
            nc.tensor.transpose(lpb[:, i * N:(i + 1) * N], pre[:, 2 * i:2 * i + 2],
                                self.ident[:N, :N])
        lab = self.small.tile([2, G * N], BF, tag="lab")
        nc.scalar.activation(lab[:], lpb[:], actf.Copy)
        la2 = [lab[:, i * N:(i + 1) * N] for i in range(G)]
        # ra = [1; qn/2] = swap of la rows: build from same columns reversed
        pre2 = self.small.tile([N, 2 * G], BF, tag="pre2")
        for i in range(G):
            nc.vector.tensor_copy(pre2[:, 2 * i:2 * i + 1], self.ones_col[:N, :])
            nc.vector.tensor_copy(pre2[:, 2 * i + 1:2 * i + 2], qn2[:, i:i + 1])
        rpb_ = self.psum_s.tile([2, G * N], BF, tag="ps")
        for i in range(G):
            nc.tensor.transpose(rpb_[:, i * N:(i + 1) * N], pre2[:, 2 * i:2 * i + 2],
                                self.ident[:N, :N])
        rab = self.small.tile([2, G * N], BF, tag="rab")
        nc.scalar.activation(rab[:], rpb_[:], actf.Copy)
        ra2 = [rab[:, i * N:(i + 1) * N] for i in range(G)]
        # ---- XcT tiles + gram2 (with +DIAG_PEN/2 diag via nid)
        m2 = self.psum_g.tile([N, GW], F32, tag="pg")
        for i in range(G):
            sl = slice(i * N, (i + 1) * N)
            ctp = self.psum_g.tile([128, 5 * N], BF, tag="pg")
            for k in range(5):
                nc.tensor.transpose(ctp[:, k * N:(k + 1) * N],
                                    xc[:, i * DIM + 128 * k:i * DIM + 128 * (k + 1)],
                                    self.ident[:N, :N])
            ct = self.xin.tile([128, 5 * N], BF, tag="xct")
            nc.vector.tensor_copy(ct[:], ctp[:])
            for k in range(5):
                nc.tensor.matmul(m2[:, sl], ct[:, k * N:(k + 1) * N],
                                 ct[:, k * N:(k + 1) * N],
                                 start=(k == 0), stop=False)
            nc.tensor.matmul(m2[:, sl], la2[i], ra2[i],
                             start=False, stop=False)
            nc.tensor.matmul(m2[:, sl], self.eyeN_bf_lhsT(), self.nid[:],
                             start=False, stop=True)
        nc.scalar.activation(m2a_sl, m2[:], actf.Copy)     # bf16 store
        self.tap("m2a", m2a_sl, r0)
        exp2 = self.gwork.tile([N, GW], F32, tag="a1")
        nc.scalar.activation(exp2[:], m2[:], actf.Exp, scale=2.0 * LAM)
        # ---- top-k threshold + W + M
        w2 = self.gwork.tile([N, GW], F32, tag="gw")
        rs2 = self.small.tile([N, G], F32, tag="rs2")
        for i in range(G):
            sl = slice(i * N, (i + 1) * N)
            m8 = self.small.tile([N, 8], F32, tag="m8")
            cp2 = self.gwork.tile([N, N], F32, tag="cp2")
            cp3 = self.gwork.tile([N, N], F32, tag="cp3")
            nc.vector.max(m8[:], exp2[:, sl])
            nc.vector.match_replace(cp2[:], m8[:], exp2[:, sl], -1.0)
            nc.vector.max(m8[:], cp2[:])
            nc.vector.match_replace(cp3[:], m8[:], cp2[:], -1.0)
            m8c = self.small.tile([N, 8], F32, tag="m8c")
            nc.vector.max(m8c[:], cp3[:])
            nc.vector.scalar_tensor_tensor(w2[:, sl], exp2[:, sl], m8c[:, 3:4],
                                           exp2[:, sl], alu.is_ge, alu.mult,
                                           accum_out=rs2[:, i:i + 1])
        rows_p2, rows_s2 = self.degree_rows(rs2, want_scaled=True)
        op2 = self.psum_g.tile([N, GW], F32, tag="pg")
        for i in range(G):
            nc.tensor.matmul(op2[:, i * N:(i + 1) * N], rows_p2[i], rows_s2[i])
        mmb = self.gwork.tile([N, GW], BF, tag="mmb")
        nc.vector.tensor_tensor(mmb[:], op2[:], w2[:], alu.mult)   # alpha*W
        self.tap("mm", mmb[:, 0:N], r0)
        # ---- Newton (grouped)
        mt = self.gwork.tile([N, GW], BF, tag="mt")
        mtp = self.psum_g.tile([N, GW], BF, tag="pg")
        for i in range(G):
            nc.tensor.transpose(mtp[:, i * N:(i + 1) * N],
                                mmb[:, i * N:(i + 1) * N], self.ident[:N, :N])
        nc.scalar.activation(mt[:], mtp[:], actf.Copy)
        r3 = lambda t: t.rearrange("p (g n) -> p g n", n=N)
        bb = self.gwork.tile([N, GW], BF, tag="bb")
        nc.vector.scalar_tensor_tensor(r3(bb[:]), r3(mmb[:]), -1.0,
                                       bc3(self.eyeN[:], G), alu.mult, alu.add)
        yt = self.gwork.tile([N, GW], BF, tag="yt")
        nc.vector.scalar_tensor_tensor(r3(yt[:]), r3(mmb[:]), 1.0,
                                       bc3(self.eyeN[:], G), alu.mult, alu.add)
        y = self.gwork.tile([N, GW], BF, tag="y")
        nc.vector.scalar_tensor_tensor(r3(y[:]), r3(mt[:]), 1.0,
                                       bc3(self.eyeN[:], G), alu.mult, alu.add)
        for it in range(NEWTON_ITERS):
            last = it == NEWTON_ITERS - 1
            tp = self.psum_g.tile([N, GW], F32, tag="pg")
            for i in range(G):
                sl = slice(i * N, (i + 1) * N)
                nc.tensor.matmul(tp[:, sl], bb[:, sl], y[:, sl])
            u = self.gwork.tile([N, GW], BF, tag="u")
            nc.vector.scalar_tensor_tensor(r3(u[:]), r3(tp[:]), -1.0,
                                           bc3(self.twoI[:], G), alu.mult, alu.add)
            ynp = self.psum_g.tile([N, GW], F32, tag="pg")
            for i in range(G):
                sl = slice(i * N, (i + 1) * N)
                nc.tensor.matmul(ynp[:, sl], yt[:, sl], u[:, sl])
            if last:
                nc.scalar.activation(y_sl, ynp[:], actf.Copy)
            else:
                yn = self.gwork.tile([N, GW], BF, tag="y")
                nc.scalar.activation(yn[:], ynp[:], actf.Copy)
                sp = self.psum_g.tile([N, GW], F32, tag="pg")
                for i in range(G):
                    sl = slice(i * N, (i + 1) * N)
                    nc.tensor.matmul(sp[:, sl], y[:, sl], bb[:, sl])
                ut = self.gwork.tile([N, GW], BF, tag="u2")
                nc.vector.scalar_tensor_tensor(r3(ut[:]), r3(sp[:]), -1.0,
                                               bc3(self.twoI[:], G), alu.mult,
                                               alu.add)
                ytp = self.psum_g.tile([N, GW], F32, tag="pg")
                for i in range(G):
                    sl = slice(i * N, (i + 1) * N)
                    nc.tensor.matmul(ytp[:, sl], ut[:, sl], yt[:, sl])
                ytn = self.gwork.tile([N, GW], BF, tag="yt")
                nc.scalar.activation(ytn[:], ytp[:], actf.Copy)
                y, yt = yn, ytn
        self.tap("y", y_sl, r0)

    _eyeN_bf = None

    def eyeN_bf_lhsT(self):
        if self._eyeN_bf is None:
            self._eyeN_bf = self.ident[:N, :N]
        return self._eyeN_bf

    # ---------------------------------------------------------------- phase 2
    def colsum_mm(self, src_ap, n_rows):
        nc = self.nc
        ones = self.ones_colf if src_ap.dtype == F32 else self.ones_col
        cs = self.psum_2.tile([1, self.BW], F32, tag="p2")
        nc.tensor.matmul(cs[:], ones[:n_rows, :], src_ap)
        return cs

    def bcast_mm(self, row_ap):
        nc = self.nc
        bc = self.psum_2.tile([N, self.BW], F32, tag="p2")
        ones = self.ones_rowf if row_ap.dtype == F32 else self.ones_row
        nc.tensor.matmul(bc[:], ones[:, :N], row_ap)
        return bc

    def sinkhorn(self, P, n_rows, c_val, clamp):
        nc, B, BW = self.nc, self.B, self.BW
        for _ in range(SINK_ITERS):
            u = self.otp.tile([n_rows, B], F32, tag="u")
            p3 = P[0:n_rows, :].rearrange("p (r w) -> p r w", w=WAYS)
            nc.vector.tensor_reduce(u[:], p3, axl.X, alu.add)
            ui = self.otp.tile([n_rows, B], F32, tag="ui")
            self.recip(ui[:], u[:])
            uib = ui[:].unsqueeze(2).broadcast_to((n_rows, B, WAYS))
            nc.vector.tensor_tensor(p3, p3, uib, alu.mult)
            cs = self.colsum_mm(P[0:n_rows, :], n_rows)
            cf = self.otp.tile([1, BW], F32, tag="cf")
            self.recip(cf[:], cs[:])
            bc = self.bcast_mm(cf[:])
            nc.vector.scalar_tensor_tensor(P[0:n_rows, :], bc[0:n_rows, :],
                                           c_val, P[0:n_rows, :],
                                           alu.mult, alu.mult)
            if clamp:
                nc.vector.scalar_tensor_tensor(P[:], P[:], self.qmask[:],
                                               self.ohp[:], alu.mult, alu.add)

    def dist_exp(self, zt, m2a_store, P, t5pack):
        nc, B, BW = self.nc, self.B, self.BW
        t5p = self.psum_2.tile([N, BW], F32, tag="p2")
        for i in range(B):
            nc.tensor.matmul(t5p[:, i * WAYS:(i + 1) * WAYS],
                             m2a_store[:, i * N:(i + 1) * N],
                             zt[:, i * WAYS:(i + 1) * WAYS])
        h = self.otp.tile([N, BW], F32, tag="h")
        nc.vector.tensor_tensor(h[:], t5p[:], zt[:], alu.mult)
        zmz = self.colsum_mm(h[:], N)
        epn = self.otp.tile([1, BW], F32, tag="epn")
        nc.scalar.activation(epn[:], zmz[:], actf.Exp, scale=-LAM)
        nc.scalar.activation(P[0:QS, :], t5p[0:QS, :], actf.Exp,
                             scale=2.0 * LAM)
        bc = self.bcast_mm(epn[:])
        nc.vector.tensor_tensor(P[0:QS, :], P[0:QS, :], bc[0:QS, :], alu.mult)

    def phase2_batch(self, b, m2a_store, y_store):
        nc, d, B, BW = self.nc, self.d, self.B, self.BW
        yq = self.otp.tile([QS, BW], BF, tag="yq")
        nc.sync.dma_start(yq[:], d["yqp"][b])
        P = self.otp.tile([N, BW], BF, tag="P")
        nc.vector.tensor_copy(P[:], self.ohp[:])
        zt = self.otp.tile([N, BW], BF, tag="zt")
        nc.vector.tensor_copy(zt[:], self.msp[:])
        t5pack = None
        for ep in range(EPOCHS):
            self.dist_exp(zt, m2a_store, P, t5pack)
            self.sinkhorn(P, QS, float(QS // WAYS), clamp=False)
            zap = self.psum_2.tile([N, BW], F32, tag="p2")
            for i in range(B):
                nc.tensor.matmul(zap[:, i * WAYS:(i + 1) * WAYS],
                                 y_store[:, i * N:(i + 1) * N],
                                 P[:, i * WAYS:(i + 1) * WAYS])
            nc.scalar.activation(P[:], zap[:], actf.Relu)
            self.sinkhorn(P, N, float(N // WAYS), clamp=True)
            csz = self.colsum_mm(P[:], N)
            ci = self.otp.tile([1, BW], F32, tag="cf")
            self.recip(ci[:], csz[:])
            bcz = self.bcast_mm(ci[:])
            t = self.otp.tile([N, BW], F32, tag="h")
            nc.vector.scalar_tensor_tensor(t[:], bcz[:], UR, P[:],
                                           alu.mult, alu.mult)
            ztn = self.otp.tile([N, BW], BF, tag="zt")
            nc.vector.scalar_tensor_tensor(ztn[:], zt[:], 1.0 - UR, t[:],
                                           alu.mult, alu.add)
            zt = ztn
        self.dist_exp(zt, m2a_store, P, t5pack)
        self.sinkhorn(P, QS, float(QS // WAYS), clamp=False)
        if "pfin" in self.debug:
            self.tap("pfin", P[:], b)
        pt = self.otp.tile([QS, BW], F32, tag="pt")
        nc.vector.tensor_tensor(pt[:], P[0:QS, :], yq[:], alu.mult)
        ptr = self.otp.tile([QS, B], F32, tag="ptr")
        nc.vector.tensor_reduce(ptr[:], pt[:].rearrange("p (r w) -> p r w", w=WAYS),
                                axl.X, alu.add)
        pmx = self.otp.tile([QS, B], F32, tag="pmx")
        nc.vector.tensor_reduce(pmx[:], P[0:QS, :].rearrange("p (r w) -> p r w", w=WAYS),
                                axl.X, alu.max)
        ok = self.otp.tile([QS, B], BF, tag="ok")
        nc.vector.tensor_tensor(ok[:], ptr[:], pmx[:], alu.is_ge)
        am = self.psum_2.tile([1, B], F32, tag="p2")
        nc.tensor.matmul(am[:], self.ones_col[:QS, :], ok[:])
        accs = self.otp.tile([1, B], F32, tag="accs")
        nc.scalar.activation(accs[:], am[:], actf.Copy, scale=1.0 / QS)
        nc.sync.dma_start(d["acc"][b * B:(b + 1) * B].unsqueeze(0), accs[:])

    def run_all(self, R, NB):
        B = self.B
        for b in range(NB):
            m2a_store = self.store.tile([N, B * N], BF, tag="m2a_store")
            y_store = self.store.tile([N, B * N], BF, tag="y_store")
            for g in range(B // G):
                r0 = b * B + g * G
                self.phase1_group(r0,
                                  m2a_store[:, g * G * N:(g + 1) * G * N],
                                  y_store[:, g * G * N:(g + 1) * G * N])
            self.phase2_batch(b, m2a_store, y_store)


def build(R, B, num_devices=8, debug=(), trn="TRN2"):
    NB = R // B
    assert NB * B == R and B % G == 0
    nc = bacc.Bacc(trn, target_bir_lowering=False, debug=False,
                   enable_asserts=True, num_devices=num_devices)
    d = declare_dram(nc, R, NB, B)
    with tile.TileContext(nc) as tc:
        with ExitStack() as ctx:
            k = Kernel(tc, ctx, d, B, debug=debug)
            k.run_all(R, NB)
    nc.compile()
    return nc, d, k.dbg_tensors


# ----------------------------------------------------------------- entry point
_CACHE = {}

N_CORES = 8
R_TOTAL = 1000
R_CORE = R_TOTAL // N_CORES      # 125
BATCH = 25


def kernel(xs, xq, ys, yq):
    """Full inputs in, full output out. xs [1000,25,640] f32, xq [1000,75,640]
    f32, ys [1000,25] i32, yq [1000,75] i32 -> acc [1000] f32."""
    from concourse import bass_utils

    xs = np.asarray(xs, dtype=np.float32)
    xq = np.asarray(xq, dtype=np.float32)
    yq = np.asarray(yq, dtype=np.int32)

    if "nc" not in _CACHE:
        _CACHE["nc"] = build(R_CORE, BATCH, num_devices=N_CORES)[0]
    nc = _CACHE["nc"]

    in_maps = []
    for c in range(N_CORES):
        sl = slice(c * R_CORE, (c + 1) * R_CORE)
        in_maps.append(host_inputs(xs[sl], xq[sl], yq[sl],
                                   R_CORE // BATCH, BATCH))
    res = bass_utils.run_bass_kernel_spmd(nc, in_maps,
                                          core_ids=list(range(N_CORES)))
    return np.concatenate([res.results[c]["acc"] for c in range(N_CORES)])



# revision 38
# speedup vs baseline: 1.0307x; 1.0307x over previous
"""Trainium2 Bass kernel for nn_DataManifolds_18915035972342 (gnn_message_passing).

Self-contained: builds an 8-core SPMD Bass/Tile program on first call,
shards the 1000 runs across 8 NeuronCores (125 runs each, batches of 25),
runs the full per-run pipeline on-device, and gathers the per-run accuracy.

Per run (n=100 nodes = 75 queries + 25 support, permuted queries-first so all
engine ops start at partition 0; d=640):
  A1 = exp(-lam*sqdist(X,X));  Xc = ((I + D^-.5 A1 D^-.5)/2)^4 @ X   (elemwise ^4)
  A2 = exp(-lam*sqdist(Xc,Xc)) with diagonal killed; W = row-top-20(A2),
       symmetrically normalized;  Y = (I - alpha*W)^-T  (Newton, bf16 matmuls)
  5 epochs of { dist->exp->Sinkhorn(2) -> Y@Z -> relu -> clamped Sinkhorn(2)
                -> proto update } ; final argmax accuracy per run.

Implementation highlights:
- sqdist via one PE accumulation chain per run: 5 K-tiles of X^T (bf16) plus
  an augmented K=2 tile carrying (-qn/2_i - qn/2_j), plus a K=100 identity
  tile adding a -25 diagonal penalty (kills the diagonal after exp).
- All proto math in z-space (proto = Xc^T z): epochs touch only [100,5]
  mms against the stored dist-gram, never the [*,640] features.
- Top-20 threshold via DVE max8/match_replace (3 passes -> 20th largest).
- Degree^-1/2 via DVE fast-inverse-sqrt bit trick (keeps ACT on a single
  exp/copy/relu table set -- no ACT_TABLE_LOAD churn).
- Phase-1 processed in groups of 5 runs sharing [100,500] tiles/PSUM banks;
  phase-2 (Sinkhorn) packs 25 runs per instruction ([100,125] tiles).
"""

import numpy as np
from contextlib import ExitStack

import concourse.bass as bass
import concourse.tile as tile
from concourse import bacc, mybir

alu = mybir.AluOpType
actf = mybir.ActivationFunctionType
axl = mybir.AxisListType
BF = mybir.dt.bfloat16
F32 = mybir.dt.float32

N, NS, QS, WAYS, DIM = 100, 25, 75, 5, 640
LAM, ALPHA, UR, EPOCHS = 10.0, 0.7, 0.6, 5
SINK_ITERS = 2
NEWTON_ITERS = 3
DIAG_PEN = 50.0
G = 5                       # phase-1 group size
LN_ALPHA = float(np.log(ALPHA))
USE_RECIP_APPROX = True


def node_perm():
    return np.concatenate([np.arange(NS, N), np.arange(NS)])


def host_consts(B):
    import ml_dtypes
    bf = ml_dtypes.bfloat16
    eye = np.eye(128, dtype=np.float32)
    ys = np.repeat(np.arange(WAYS), NS // WAYS)
    ms = np.zeros((N, WAYS), np.float32)
    ms[QS + np.arange(NS), ys] = 1.0 / (NS // WAYS)
    oh = np.zeros((NS, WAYS), np.float32)
    oh[np.arange(NS), ys] = 1.0
    return {
        "ident": eye.astype(bf),
        "eyeN": np.eye(N, dtype=np.float32),
        "negEyeN": (-np.eye(N)).astype(np.float32),
        "halfI": (0.5 * np.eye(N)).astype(np.float32),
        "twoI": (2.0 * np.eye(N)).astype(np.float32),
        "twoIbf": (2.0 * np.eye(N)).astype(bf),
        "nid": (-0.5 * DIAG_PEN * np.eye(N)).astype(bf),
        "msp": np.tile(ms, (1, B)).astype(bf),
        "ohp": np.tile(oh, (1, B)).astype(bf),
        "ones_col": np.ones((128, 1), np.float32).astype(bf),
        "ones_row": np.ones((1, 128), np.float32).astype(bf),
        "ones_colf": np.ones((128, 1), np.float32),
        "ones_rowf": np.ones((1, 128), np.float32),
        "qmask": np.concatenate([np.ones((QS, 1), np.float32),
                                 np.zeros((NS, 1), np.float32)]),
        "onesw": np.ones((1, G * N), np.float32).astype(bf),
    }


def host_inputs(xs, xq, yq, NB, B):
    import ml_dtypes
    bf = ml_dtypes.bfloat16
    R = xs.shape[0]
    assert NB * B == R
    NG = R // G
    perm = node_perm()
    feat = np.concatenate([xs, xq], axis=1)[:, perm, :]
    x0 = np.ascontiguousarray(feat).astype(bf)
    xt = np.swapaxes(x0, 1, 2)                           # [R, DIM, N]
    # pre-tiled group layouts: one DMA per group for each of x0/xt/aug
    x0g = np.ascontiguousarray(
        x0.reshape(NG, G, N, DIM).transpose(0, 2, 1, 3)
    ).reshape(NG, N, G * DIM)
    xtg = np.ascontiguousarray(
        xt.reshape(NG, G, 5, 128, N).transpose(0, 3, 2, 1, 4)
    ).reshape(NG, 128, 5 * G * N)
    # gram1 aug rows from the bf16 values (consistent with PE products)
    x0f = x0.astype(np.float32)
    qn_h = -0.5 * (x0f * x0f).sum(2)                     # [R,100]
    aug1 = np.zeros((R, 4, N), np.float32)
    aug1[:, 0] = qn_h
    aug1[:, 1] = 1.0
    aug1[:, 2] = 1.0
    aug1[:, 3] = qn_h
    augg = np.ascontiguousarray(
        aug1.astype(bf).reshape(NG, G, 4, N).transpose(0, 2, 1, 3)
    ).reshape(NG, 4, G * N)
    yq1 = (yq[:, :, None] == np.arange(WAYS)[None, None, :]).astype(np.float32)
    yqp = np.ascontiguousarray(
        yq1.reshape(NB, B, QS, WAYS).transpose(0, 2, 1, 3)
    ).reshape(NB, QS, B * WAYS).astype(bf)
    out = {"x0g": x0g, "xtg": xtg, "augg": augg, "yqp": yqp}
    out.update(host_consts(B))
    return out


def declare_dram(nc, R, NB, B):
    BW = B * WAYS
    NG = R // G
    mk = lambda n, s, dt, k="ExternalInput": nc.dram_tensor(n, s, dt, kind=k).ap()
    return {
        "x0g": mk("x0g", [NG, N, G * DIM], BF),
        "xtg": mk("xtg", [NG, 128, 5 * G * N], BF),
        "augg": mk("augg", [NG, 4, G * N], BF),
        "yqp": mk("yqp", [NB, QS, BW], BF),
        "ident": mk("ident", [128, 128], BF),
        "eyeN": mk("eyeN", [N, N], F32),
        "negEyeN": mk("negEyeN", [N, N], F32),
        "halfI": mk("halfI", [N, N], F32),
        "twoI": mk("twoI", [N, N], F32),
        "twoIbf": mk("twoIbf", [N, N], BF),
        "nid": mk("nid", [N, N], BF),
        "msp": mk("msp", [N, BW], BF),
        "ohp": mk("ohp", [NS, BW], BF),
        "ones_col": mk("ones_col", [128, 1], BF),
        "ones_row": mk("ones_row", [1, 128], BF),
        "ones_colf": mk("ones_colf", [128, 1], F32),
        "ones_rowf": mk("ones_rowf", [1, 128], F32),
        "qmask": mk("qmask", [N, 1], F32),
        "onesw": mk("onesw", [1, G * N], BF),
        "acc": mk("acc", [R], F32, "ExternalOutput"),
    }


def bc3(ap, g):
    """[100,100] const -> broadcast view [100, g, 100] (step-0 middle dim)."""
    return ap.unsqueeze(1).broadcast_to((ap.shape[0], g, ap.shape[1]))


class Kernel:
    def __init__(self, tc, ctx, d, B, debug=()):
        self.tc, self.ctx, self.d, self.B = tc, ctx, d, B
        self.BW = B * WAYS
        self.nc = tc.nc
        self.debug = set(debug)
        self.dbg_tensors = {}
        p = lambda name, bufs, **kw: ctx.enter_context(
            tc.tile_pool(name=name, bufs=bufs, **kw))
        self.consts = p("consts", 1)
        self.xin = p("xin", 3)
        self.gwork = p("gwork", 3)
        self.small = p("small", 4)
        self.store = p("store", 2)
        self.otp = p("otp", 3)
        # PSUM budget (8 banks x 2KB):
        #   gram (m1/m2 [100,500]f32)          bufs=2 -> 2 banks
        #   pf   (op/op2/newton [100,500]f32)  bufs=3 -> 3 banks
        #   bf   (ctp/mtp/rows [<=128,500]bf)  bufs=2 -> 2 banks
        #   p2   (phase2 [100,125]f32)         bufs=1 -> 1 bank
        self.psum_gram = p("psum_gram", 2, space="PSUM")
        self.psum_f = p("psum_f", 2, space="PSUM")
        self.psum_bf = p("psum_bf", 2, space="PSUM")
        self.psum_2 = p("psum_2", 2, space="PSUM")
        self._load_consts()

    def _load_consts(self):
        nc, d = self.nc, self.d
        P = self.consts
        for nm, shape, dt in [
            ("ident", [128, 128], BF), ("eyeN", [N, N], F32),
            ("negEyeN", [N, N], F32),
            ("halfI", [N, N], F32), ("twoI", [N, N], F32),
            ("twoIbf", [N, N], BF),
            ("nid", [N, N], BF),
            ("msp", [N, self.BW], BF),
            ("ones_col", [128, 1], BF), ("ones_row", [1, 128], BF),
            ("ones_colf", [128, 1], F32), ("ones_rowf", [1, 128], F32),
            ("qmask", [N, 1], F32),
            ("onesw", [1, G * N], BF),
        ]:
            t = P.tile(shape, dt, tag=nm)
            nc.sync.dma_start(t[:], d[nm][:])
            setattr(self, nm, t)
        self.rsqmagic = P.tile([128, 8], mybir.dt.uint32, tag="rsqmagic")
        nc.vector.memset(self.rsqmagic[:], 0x5F3759DF)
        self.ohp = P.tile([N, self.BW], BF, tag="ohp")
        nc.vector.memset(self.ohp[0:QS, :], 0.0)
        nc.sync.dma_start(self.ohp[QS:N, :], d["ohp"][:])

    def tap(self, name, ap, r):
        if name not in self.debug:
            return
        nc = self.nc
        key = f"dbg_{name}_{r}"
        t = nc.dram_tensor(key, list(ap.shape), ap.dtype, kind="ExternalOutput").ap()
        self.dbg_tensors[key] = t
        if ap.space == bass.MemorySpace.PSUM:
            s = self.gwork.tile(list(ap.shape), ap.dtype, tag="dbgcpy")
            nc.vector.tensor_copy(s[:], ap)
            ap = s[:]
        nc.sync.dma_start(t, ap)

    def recip(self, out, in_):
        if USE_RECIP_APPROX:
            self.nc.vector.reciprocal_approx_fast(out, in_)
        else:
            self.nc.vector.reciprocal(out, in_)

    # ---------------------------------------------------------------- phase 1
    def degree_rows(self, rs_g, want_scaled):
        """rs_g [100,G] f32 rowsums -> per-run [1,100] bf16 rows of rs^-0.5
        (and alpha*rs^-0.5 if want_scaled). DVE-only fast inverse sqrt
        (bit trick + 2 Newton steps, ~5e-6 rel err) — keeps the ACT table
        set at {exp, copy, relu} for the whole program."""
        nc = self.nc
        U32 = mybir.dt.uint32
        iv = self.small.tile([N, G], U32, tag="rsq_i")
        nc.vector.tensor_scalar(iv[:], rs_g[:].bitcast(U32), 1, None,
                                alu.logical_shift_right)
        nc.vector.tensor_tensor(iv[:], self.rsqmagic[:N, :G], iv[:],
                                alu.subtract)
        yv = self.small.tile([N, G], F32, tag="rsq_y")
        tv = self.small.tile([N, G], F32, tag="rsq_t")
        nc.vector.tensor_copy(yv[:], iv[:].bitcast(F32))
        for _ in range(2):
            nc.vector.tensor_tensor(tv[:], yv[:], yv[:], alu.mult)
            nc.vector.tensor_tensor(tv[:], tv[:], rs_g[:], alu.mult)
            nc.vector.tensor_scalar(tv[:], tv[:], -0.5, 1.5, alu.mult, alu.add)
            nc.vector.tensor_tensor(yv[:], yv[:], tv[:], alu.mult)
        dm_p = self.small.tile([N, G], BF, tag="dm_p")
        nc.vector.tensor_copy(dm_p[:], yv[:])
        dm_s = None
        if want_scaled:
            dm_s = self.small.tile([N, G], BF, tag="dm_s")
            nc.vector.tensor_scalar(dm_s[:], yv[:], ALPHA, None, alu.mult)
        ppb = self.psum_bf.tile([128, G * N], BF, tag="bf")
        for i in range(G):
            nc.tensor.transpose(ppb[0:1, i * N:(i + 1) * N], dm_p[:, i:i + 1],
                                self.ident[:N, :N])
        rpb = self.small.tile([1, G * N], BF, tag="rpb")
        nc.scalar.activation(rpb[:], ppb[0:1, :], actf.Copy)
        rows_p = [rpb[:, i * N:(i + 1) * N] for i in range(G)]
        rows_s = []
        if want_scaled:
            spb = self.psum_bf.tile([128, G * N], BF, tag="bf")
            for i in range(G):
                nc.tensor.transpose(spb[0:1, i * N:(i + 1) * N], dm_s[:, i:i + 1],
                                    self.ident[:N, :N])
            rsb = self.small.tile([1, G * N], BF, tag="rsb")
            nc.scalar.activation(rsb[:], spb[0:1, :], actf.Copy)
            rows_s = [rsb[:, i * N:(i + 1) * N] for i in range(G)]
        return rows_p, rows_s

    def stage1(self, gi):
        """Chunk gi: gram1 -> a1 -> degree -> G^4. Returns (x0, g4)."""
        nc, d = self.nc, self.d
        GW = G * N
        r3 = lambda t: t.rearrange("p (g n) -> p g n", n=N)
        x0 = self.xin.tile([N, G * DIM], BF, tag="x0", bufs=6)
        nc.sync.dma_start(x0[:], d["x0g"][gi])
        xt = self.xin.tile([128, 5 * GW], BF, tag="xt", bufs=4)
        nc.sync.dma_start(xt[:], d["xtg"][gi])
        augl = self.xin.tile([2, GW], BF, tag="augl")
        nc.sync.dma_start(augl[:], d["augg"][gi, 0:2])
        augr = self.xin.tile([2, GW], BF, tag="augr")
        nc.sync.dma_start(augr[:], d["augg"][gi, 2:4])
        xts = lambda k, i: xt[:, (k * G + i) * N:(k * G + i + 1) * N]
        # gram1 (halved): m1' = qn/2_i + qn/2_j - x_i.x_j
        m1 = self.psum_gram.tile([N, GW], F32, tag="gram")
        for i in range(G):
            sl = slice(i * N, (i + 1) * N)
            for k in range(5):
                nc.tensor.matmul(m1[:, sl], xts(k, i), xts(k, i),
                                 start=(k == 0), stop=False)
            nc.tensor.matmul(m1[:, sl], augl[:, sl], augr[:, sl],
                             start=False, stop=True)
        a1 = self.gwork.tile([N, GW], F32, tag="a1")
        nc.scalar.activation(a1[:], m1[:], actf.Exp, scale=2.0 * LAM)
        self.tap("a1", a1[:], gi)
        rs1 = self.small.tile([N, G], F32, tag="rs1")
        nc.vector.tensor_reduce(rs1[:], r3(a1[:]), axl.X, alu.add)
        rows_p, _ = self.degree_rows(rs1, want_scaled=False)
        op = self.psum_f.tile([N, GW], F32, tag="pf")
        for i in range(G):
            nc.tensor.matmul(op[:, i * N:(i + 1) * N], rows_p[i], rows_p[i])
        gw = self.gwork.tile([N, GW], F32, tag="gw")
        nc.vector.tensor_tensor(gw[:], op[:], a1[:], alu.mult)
        gh = self.gwork.tile([N, GW], F32, tag="gh")
        nc.gpsimd.tensor_tensor(r3(gh[:]), r3(gw[:]),
                                bc3(self.eyeN[:], G), alu.add)
        gh2 = self.gwork.tile([N, GW], F32, tag="gh2")
        nc.gpsimd.tensor_tensor(gh2[:], gh[:], gh[:], alu.mult)
        g4 = self.gwork.tile([N, GW], BF, tag="g4", bufs=6)
        nc.gpsimd.tensor_tensor(g4[:], gh2[:], gh2[:], alu.mult)
        # g4 = (2*Gh)^4 = 16*Gh^4; the 1/16 is folded into the ct copy scale
        self.tap("g4", g4[:], gi)
        return x0, g4

    def stage2(self, x0, g4, m2a_sl):
        """Chunk: conv (directly transposed), gram2 + aug + diag penalty,
        m2a store, exp2, top-k -> (w2, rs2).

        XcT = X0^T @ G4 exploits G4's symmetry: no transposes and no
        untransposed Xc at all; qn2 comes from a mid-chain diag read."""
        nc = self.nc
        GW = G * N
        r3 = lambda t: t.rearrange("p (g n) -> p g n", n=N)
        m2 = self.psum_gram.tile([N, GW], F32, tag="gram")
        labq = self.small.tile([1, G * N], BF, tag="labq")
        for i in range(G):
            sl = slice(i * N, (i + 1) * N)
            ctp = self.psum_f.tile([128, 5 * N], F32, tag="pf")
            for k in range(5):
                nc.tensor.matmul(ctp[:, k * N:(k + 1) * N],
                                 x0[:, i * DIM + 128 * k:i * DIM + 128 * (k + 1)],
                                 g4[:, i * N:(i + 1) * N])
            ct = self.xin.tile([128, 5 * N], BF, tag="xct", bufs=6)
            nc.scalar.activation(ct[:], ctp[:], actf.Copy, scale=1.0 / 16.0)
            self.tap("ct", ct[:], self._dbg_c * 10 + i)
            # squared node norms via ones-vector colsum of ct.^2 (per run row)
            sq = self.xin.tile([128, 5 * N], F32, tag="sq", bufs=6)
            nc.vector.tensor_tensor(sq[:], ct[:], ct[:], alu.mult)
            qp = self.psum_f.tile([1, 5 * N], F32, tag="pf")
            for k in range(5):
                nc.tensor.matmul(qp[0:1, 0:N], self.ones_colf[:, :],
                                 sq[:, k * N:(k + 1) * N],
                                 start=(k == 0), stop=(k == 4))
            nc.scalar.activation(labq[0:1, sl], qp[0:1, 0:N], actf.Copy,
                                 scale=-0.5)
            # gram2 for this run, then aug rank-1s + diag penalty; each
            # run's chain closes before the next one starts (PSUM start
            # bits are bank-granular)
            for k in range(5):
                nc.tensor.matmul(m2[:, sl], ct[:, k * N:(k + 1) * N],
                                 ct[:, k * N:(k + 1) * N],
                                 start=(k == 0), stop=False)
            nc.tensor.matmul(m2[:, sl], labq[0:1, sl], self.onesw[0:1, sl],
                             start=False, stop=False)
            nc.tensor.matmul(m2[:, sl], self.onesw[0:1, sl], labq[0:1, sl],
                             start=False, stop=False)
            nc.tensor.matmul(m2[:, sl], self.eyeN_bf_lhsT(), self.nid[:],
                             start=False, stop=True)
        nc.scalar.activation(m2a_sl, m2[:], actf.Copy)     # bf16 store
        self.tap("m2a", m2a_sl, self._dbg_c)
        exp2 = self.gwork.tile([N, GW], F32, tag="exp2")
        nc.scalar.activation(exp2[:], m2[:], actf.Exp, scale=2.0 * LAM)
        # top-k threshold + masked W rows
        w2 = self.gwork.tile([N, GW], F32, tag="w2", bufs=6)
        rs2 = self.small.tile([N, G], F32, tag="rs2", bufs=6)
        for i in range(G):
            sl = slice(i * N, (i + 1) * N)
            m8 = self.small.tile([N, 8], F32, tag="m8")
            cp2 = self.gwork.tile([N, N], F32, tag="cp2")
            cp3 = self.gwork.tile([N, N], F32, tag="cp3")
            nc.vector.max(m8[:], exp2[:, sl])
            nc.vector.match_replace(cp2[:], m8[:], exp2[:, sl], -1.0)
            nc.vector.max(m8[:], cp2[:])
            nc.vector.match_replace(cp3[:], m8[:], cp2[:], -1.0)
            m8c = self.small.tile([N, 8], F32, tag="m8c")
            nc.vector.max(m8c[:], cp3[:])
            nc.vector.scalar_tensor_tensor(w2[:, sl], exp2[:, sl], m8c[:, 3:4],
                                           exp2[:, sl], alu.is_ge, alu.mult,
                                           accum_out=rs2[:, i:i + 1])
        return w2, rs2

    def stage3(self, w2, rs2, y_sl):
        """Chunk: symmetric normalize -> alpha*W -> Newton inverse -> y_sl."""
        nc = self.nc
        GW = G * N
        r3 = lambda t: t.rearrange("p (g n) -> p g n", n=N)
        self.tap("w2", w2[:], self._dbg_c)
        self.tap("rs2", rs2[:], self._dbg_c)
        rows_p2, rows_s2 = self.degree_rows(rs2, want_scaled=True)
        op2 = self.psum_f.tile([N, GW], F32, tag="pf")
        for i in range(G):
            nc.tensor.matmul(op2[:, i * N:(i + 1) * N], rows_p2[i], rows_s2[i])
        mmb = self.gwork.tile([N, GW], BF, tag="mmb")
        nc.vector.tensor_tensor(mmb[:], op2[:], w2[:], alu.mult)   # alpha*W
        self.tap("mm", mmb[:], self._dbg_c)
        # Newton; bn = M - I so the 2I-folds ride the PE accumulation
        mtp = self.psum_bf.tile([128, GW], BF, tag="bf")
        for i in range(G):
            nc.tensor.transpose(mtp[0:N, i * N:(i + 1) * N],
                                mmb[:, i * N:(i + 1) * N], self.ident[:N, :N])
        y = self.gwork.tile([N, GW], BF, tag="y")
        nc.vector.scalar_tensor_tensor(r3(y[:]), r3(mtp[0:N, :]), 1.0,
                                       bc3(self.eyeN[:], G), alu.mult, alu.add)
        bn = self.gwork.tile([N, GW], BF, tag="bn")
        nc.gpsimd.tensor_tensor(r3(bn[:]), r3(mmb[:]),
                                bc3(self.eyeN[:], G), alu.subtract)
        yt = self.gwork.tile([N, GW], BF, tag="yt")
        nc.gpsimd.tensor_tensor(r3(yt[:]), r3(mmb[:]),
                                bc3(self.eyeN[:], G), alu.add)
        for it in range(NEWTON_ITERS):
            last = it == NEWTON_ITERS - 1
            # u = 2I - B@Y = bn@y + 2I, with the 2I added by the PE chain
            tp = self.psum_f.tile([N, GW], F32, tag="pf")
            for i in range(G):
                sl = slice(i * N, (i + 1) * N)
                nc.tensor.matmul(tp[:, sl], bn[:, sl], y[:, sl],
                                 start=True, stop=False)
                nc.tensor.matmul(tp[:, sl], self.ident[:N, :N], self.twoIbf[:],
                                 start=False, stop=True)
            u = self.gwork.tile([N, GW], BF, tag="u")
            nc.vector.tensor_copy(u[:], tp[:])
            ynp = self.psum_f.tile([N, GW], F32, tag="pf")
            for i in range(G):
                sl = slice(i * N, (i + 1) * N)
                nc.tensor.matmul(ynp[:, sl], yt[:, sl], u[:, sl])
            if last:
                nc.scalar.activation(y_sl, ynp[:], actf.Copy)
            else:
                yn = self.gwork.tile([N, GW], BF, tag="y")
                nc.scalar.activation(yn[:], ynp[:], actf.Copy)
                sp = self.psum_f.tile([N, GW], F32, tag="pf")
                for i in range(G):
                    sl = slice(i * N, (i + 1) * N)
                    nc.tensor.matmul(sp[:, sl], y[:, sl], bn[:, sl])
                ut = self.gwork.tile([N, GW], BF, tag="u2")
                nc.vector.scalar_tensor_tensor(r3(ut[:]), r3(sp[:]), 1.0,
                                               bc3(self.twoI[:], G), alu.mult,
                                               alu.add)
                ytp = self.psum_f.tile([N, GW], F32, tag="pf")
                for i in range(G):
                    sl = slice(i * N, (i + 1) * N)
                    nc.tensor.matmul(ytp[:, sl], ut[:, sl], yt[:, sl])
                ytn = self.gwork.tile([N, GW], BF, tag="yt")
                nc.scalar.activation(ytn[:], ytp[:], actf.Copy)
                y, yt = yn, ytn
        self.tap("y", y_sl, self._dbg_c)

    _eyeN_bf = None

    def eyeN_bf_lhsT(self):
        if self._eyeN_bf is None:
            self._eyeN_bf = self.ident[:N, :N]
        return self._eyeN_bf

    # ---------------------------------------------------------------- phase 2
    def colsum_mm(self, src_ap, n_rows):
        nc = self.nc
        ones = self.ones_colf if src_ap.dtype == F32 else self.ones_col
        cs = self.psum_2.tile([1, self.BW], F32, tag="p2")
        nc.tensor.matmul(cs[:], ones[:n_rows, :], src_ap)
        return cs

    def bcast_mm(self, row_ap):
        nc = self.nc
        bc = self.psum_2.tile([N, self.BW], F32, tag="p2")
        ones = self.ones_rowf if row_ap.dtype == F32 else self.ones_row
        nc.tensor.matmul(bc[:], ones[:, :N], row_ap)
        return bc

    def sinkhorn(self, P, n_rows, c_val, clamp):
        nc, B, BW = self.nc, self.B, self.BW
        for _ in range(SINK_ITERS):
            u = self.otp.tile([n_rows, B], F32, tag="u")
            p3 = P[0:n_rows, :].rearrange("p (r w) -> p r w", w=WAYS)
            nc.vector.tensor_reduce(u[:], p3, axl.X, alu.add)
            ui = self.otp.tile([n_rows, B], F32, tag="ui")
            self.recip(ui[:], u[:])
            uib = ui[:].unsqueeze(2).broadcast_to((n_rows, B, WAYS))
            nc.gpsimd.tensor_tensor(p3, p3, uib, alu.mult)
            cs = self.colsum_mm(P[0:n_rows, :], n_rows)
            cf = self.otp.tile([1, BW], F32, tag="cf")
            self.recip(cf[:], cs[:])
            bc = self.bcast_mm(cf[:])
            nc.vector.scalar_tensor_tensor(P[0:n_rows, :], bc[0:n_rows, :],
                                           c_val, P[0:n_rows, :],
                                           alu.mult, alu.mult)
            if clamp:
                nc.vector.scalar_tensor_tensor(P[:], P[:], self.qmask[:],
                                               self.ohp[:], alu.mult, alu.add)

    def dist_exp(self, zt, m2a_store, P, t5pack):
        nc, B, BW = self.nc, self.B, self.BW
        t5p = self.psum_2.tile([N, BW], F32, tag="p2")
        for i in range(B):
            nc.tensor.matmul(t5p[:, i * WAYS:(i + 1) * WAYS],
                             m2a_store[:, i * N:(i + 1) * N],
                             zt[:, i * WAYS:(i + 1) * WAYS])
        h = self.otp.tile([N, BW], F32, tag="h")
        nc.vector.tensor_tensor(h[:], t5p[:], zt[:], alu.mult)
        zmz = self.colsum_mm(h[:], N)
        epn = self.otp.tile([1, BW], F32, tag="epn")
        nc.scalar.activation(epn[:], zmz[:], actf.Exp, scale=-LAM)
        nc.scalar.activation(P[0:QS, :], t5p[0:QS, :], actf.Exp,
                             scale=2.0 * LAM)
        bc = self.bcast_mm(epn[:])
        nc.vector.tensor_tensor(P[0:QS, :], P[0:QS, :], bc[0:QS, :], alu.mult)

    def phase2_batch(self, b, m2a_store, y_store):
        nc, d, B, BW = self.nc, self.d, self.B, self.BW
        yq = self.otp.tile([QS, BW], BF, tag="yq")
        nc.sync.dma_start(yq[:], d["yqp"][b])
        P = self.otp.tile([N, BW], BF, tag="P")
        nc.vector.tensor_copy(P[:], self.ohp[:])
        zt = self.otp.tile([N, BW], BF, tag="zt")
        nc.vector.tensor_copy(zt[:], self.msp[:])
        t5pack = None
        for ep in range(EPOCHS):
            self.dist_exp(zt, m2a_store, P, t5pack)
            self.sinkhorn(P, QS, float(QS // WAYS), clamp=False)
            zap = self.psum_2.tile([N, BW], F32, tag="p2")
            for i in range(B):
                nc.tensor.matmul(zap[:, i * WAYS:(i + 1) * WAYS],
                                 y_store[:, i * N:(i + 1) * N],
                                 P[:, i * WAYS:(i + 1) * WAYS])
            nc.scalar.activation(P[:], zap[:], actf.Relu)
            self.sinkhorn(P, N, float(N // WAYS), clamp=True)
            csz = self.colsum_mm(P[:], N)
            ci = self.otp.tile([1, BW], F32, tag="cf")
            self.recip(ci[:], csz[:])
            bcz = self.bcast_mm(ci[:])
            t = self.otp.tile([N, BW], F32, tag="h")
            nc.vector.scalar_tensor_tensor(t[:], bcz[:], UR, P[:],
                                           alu.mult, alu.mult)
            ztn = self.otp.tile([N, BW], BF, tag="zt")
            nc.vector.scalar_tensor_tensor(ztn[:], zt[:], 1.0 - UR, t[:],
                                           alu.mult, alu.add)
            zt = ztn
        self.dist_exp(zt, m2a_store, P, t5pack)
        self.sinkhorn(P, QS, float(QS // WAYS), clamp=False)
        if "pfin" in self.debug:
            self.tap("pfin", P[:], b)
        pt = self.otp.tile([QS, BW], F32, tag="pt")
        nc.gpsimd.tensor_tensor(pt[:], P[0:QS, :], yq[:], alu.mult)
        ptr = self.otp.tile([QS, B], F32, tag="ptr")
        nc.vector.tensor_reduce(ptr[:], pt[:].rearrange("p (r w) -> p r w", w=WAYS),
                                axl.X, alu.add)
        pmx = self.otp.tile([QS, B], F32, tag="pmx")
        nc.vector.tensor_reduce(pmx[:], P[0:QS, :].rearrange("p (r w) -> p r w", w=WAYS),
                                axl.X, alu.max)
        ok = self.otp.tile([QS, B], BF, tag="ok")
        nc.vector.tensor_tensor(ok[:], ptr[:], pmx[:], alu.is_ge)
        am = self.psum_2.tile([1, B], F32, tag="p2")
        nc.tensor.matmul(am[:], self.ones_col[:QS, :], ok[:])
        accs = self.otp.tile([1, B], F32, tag="accs")
        nc.scalar.activation(accs[:], am[:], actf.Copy, scale=1.0 / QS)
        nc.sync.dma_start(d["acc"][b * B:(b + 1) * B].unsqueeze(0), accs[:])

    def run_all(self, R, NB, repeat=1):
        def body():
            B = self.B
            NGB = B // G       # chunks per batch
            for b in range(NB):
                m2a_store = self.store.tile([N, B * N], BF, tag="m2a_store")
                y_store = self.store.tile([N, B * N], BF, tag="y_store")
                # stage-major emission: all chunks through each stage so the
                # scheduler always has independent per-chunk work in flight
                s1 = [self.stage1(b * NGB + c) for c in range(NGB)]
                s2 = []
                for c in range(NGB):
                    self._dbg_c = b * NGB + c
                    s2.append(self.stage2(s1[c][0], s1[c][1],
                                          m2a_store[:, c * G * N:(c + 1) * G * N]))
                for c in range(NGB):
                    self._dbg_c = b * NGB + c
                    self.stage3(s2[c][0], s2[c][1],
                                y_store[:, c * G * N:(c + 1) * G * N])
                self.phase2_batch(b, m2a_store, y_store)
        if repeat == 1:
            body()
        else:
            # measurement aid: execute the whole workload `repeat` times on
            # device so (t[repeat=K] - t[repeat=1])/(K-1) isolates pure HW
            # execution time from host/tunnel dispatch latency.
            with self.tc.For_i(0, repeat):
                body()


def build(R, B, num_devices=8, debug=(), trn="TRN2", repeat=1):
    NB = R // B
    assert NB * B == R and B % G == 0
    nc = bacc.Bacc(trn, target_bir_lowering=False, debug=False,
                   enable_asserts=True, num_devices=num_devices)
    d = declare_dram(nc, R, NB, B)
    with tile.TileContext(nc) as tc:
        with ExitStack() as ctx:
            k = Kernel(tc, ctx, d, B, debug=debug)
            k.run_all(R, NB, repeat=repeat)
    nc.compile()
    return nc, d, k.dbg_tensors


# ----------------------------------------------------------------- entry point
_CACHE = {}

N_CORES = 8
R_TOTAL = 1000
R_CORE = R_TOTAL // N_CORES      # 125
BATCH = 25


def kernel(xs, xq, ys, yq):
    """Full inputs in, full output out. xs [1000,25,640] f32, xq [1000,75,640]
    f32, ys [1000,25] i32, yq [1000,75] i32 -> acc [1000] f32."""
    from concourse import bass_utils

    xs = np.asarray(xs, dtype=np.float32)
    xq = np.asarray(xq, dtype=np.float32)
    yq = np.asarray(yq, dtype=np.int32)

    if "nc" not in _CACHE:
        _CACHE["nc"] = build(R_CORE, BATCH, num_devices=N_CORES)[0]
    nc = _CACHE["nc"]

    in_maps = []
    for c in range(N_CORES):
        sl = slice(c * R_CORE, (c + 1) * R_CORE)
        in_maps.append(host_inputs(xs[sl], xq[sl], yq[sl],
                                   R_CORE // BATCH, BATCH))
    res = bass_utils.run_bass_kernel_spmd(nc, in_maps,
                                          core_ids=list(range(N_CORES)))
    return np.concatenate([res.results[c]["acc"] for c in range(N_CORES)])



# revision 39
# speedup vs baseline: 55.5111x; 53.8578x over previous
"""Trainium2 Bass kernel for nn_DataManifolds_18915035972342 (gnn_message_passing).

Self-contained: builds an 8-core SPMD Bass/Tile program on first call,
shards the 1000 runs across 8 NeuronCores (125 runs each, batches of 25),
runs the full per-run pipeline on-device, and gathers the per-run accuracy.

Per run (n=100 nodes = 75 queries + 25 support, permuted queries-first so all
engine ops start at partition 0; d=640):
  A1 = exp(-lam*sqdist(X,X));  Xc = ((I + D^-.5 A1 D^-.5)/2)^4 @ X   (elemwise ^4)
  A2 = exp(-lam*sqdist(Xc,Xc)) with diagonal killed; W = row-top-20(A2),
       symmetrically normalized;  Y = (I - alpha*W)^-T  (Newton, bf16 matmuls)
  5 epochs of { dist->exp->Sinkhorn(2) -> Y@Z -> relu -> clamped Sinkhorn(2)
                -> proto update } ; final argmax accuracy per run.

Implementation highlights:
- sqdist via one PE accumulation chain per run: 5 K-tiles of X^T (bf16) plus
  an augmented K=2 tile carrying (-qn/2_i - qn/2_j), plus a K=100 identity
  tile adding a -25 diagonal penalty (kills the diagonal after exp).
- All proto math in z-space (proto = Xc^T z): epochs touch only [100,5]
  mms against the stored dist-gram, never the [*,640] features.
- Top-20 threshold via DVE max8/match_replace (3 passes -> 20th largest).
- Degree^-1/2 via DVE fast-inverse-sqrt bit trick (keeps ACT on a single
  exp/copy/relu table set -- no ACT_TABLE_LOAD churn).
- Phase-1 processed in groups of 5 runs sharing [100,500] tiles/PSUM banks;
  phase-2 (Sinkhorn) packs 25 runs per instruction ([100,125] tiles).
"""

import numpy as np
from contextlib import ExitStack

import concourse.bass as bass
import concourse.tile as tile
from concourse import bacc, mybir

alu = mybir.AluOpType
actf = mybir.ActivationFunctionType
axl = mybir.AxisListType
BF = mybir.dt.bfloat16
F32 = mybir.dt.float32

N, NS, QS, WAYS, DIM = 100, 25, 75, 5, 640
LAM, ALPHA, UR, EPOCHS = 10.0, 0.7, 0.6, 5
SINK_ITERS = 2
NEWTON_ITERS = 3
DIAG_PEN = 50.0
G = 5                       # phase-1 group size
LN_ALPHA = float(np.log(ALPHA))
USE_RECIP_APPROX = True


def node_perm():
    return np.concatenate([np.arange(NS, N), np.arange(NS)])


def host_consts(B):
    import ml_dtypes
    bf = ml_dtypes.bfloat16
    eye = np.eye(128, dtype=np.float32)
    ys = np.repeat(np.arange(WAYS), NS // WAYS)
    ms = np.zeros((N, WAYS), np.float32)
    ms[QS + np.arange(NS), ys] = 1.0 / (NS // WAYS)
    oh = np.zeros((NS, WAYS), np.float32)
    oh[np.arange(NS), ys] = 1.0
    return {
        "ident": eye.astype(bf),
        "eyeN": np.eye(N, dtype=np.float32),
        "negEyeN": (-np.eye(N)).astype(np.float32),
        "halfI": (0.5 * np.eye(N)).astype(np.float32),
        "twoI": (2.0 * np.eye(N)).astype(np.float32),
        "twoIbf": (2.0 * np.eye(N)).astype(bf),
        "nid": (-0.5 * DIAG_PEN * np.eye(N)).astype(bf),
        "msp": np.tile(ms, (1, B)).astype(bf),
        "ohp": np.tile(oh, (1, B)).astype(bf),
        "ones_col": np.ones((128, 1), np.float32).astype(bf),
        "ones_row": np.ones((1, 128), np.float32).astype(bf),
        "ones_colf": np.ones((128, 1), np.float32),
        "ones_rowf": np.ones((1, 128), np.float32),
        "qmask": np.concatenate([np.ones((QS, 1), np.float32),
                                 np.zeros((NS, 1), np.float32)]),
        "onesw": np.ones((1, G * N), np.float32).astype(bf),
    }


def host_inputs(xs, xq, yq, NB, B):
    import ml_dtypes
    bf = ml_dtypes.bfloat16
    R = xs.shape[0]
    assert NB * B == R
    NG = R // G
    perm = node_perm()
    feat = np.concatenate([xs, xq], axis=1)[:, perm, :]
    x0 = np.ascontiguousarray(feat).astype(bf)
    xt = np.swapaxes(x0, 1, 2)                           # [R, DIM, N]
    # pre-tiled group layouts: one DMA per group for each of x0/xt/aug
    x0g = np.ascontiguousarray(
        x0.reshape(NG, G, N, DIM).transpose(0, 2, 1, 3)
    ).reshape(NG, N, G * DIM)
    xtg = np.ascontiguousarray(
        xt.reshape(NG, G, 5, 128, N).transpose(0, 3, 2, 1, 4)
    ).reshape(NG, 128, 5 * G * N)
    # gram1 aug rows from the bf16 values (consistent with PE products)
    x0f = x0.astype(np.float32)
    qn_h = -0.5 * (x0f * x0f).sum(2)                     # [R,100]
    aug1 = np.zeros((R, 4, N), np.float32)
    aug1[:, 0] = qn_h
    aug1[:, 1] = 1.0
    aug1[:, 2] = 1.0
    aug1[:, 3] = qn_h
    augg = np.ascontiguousarray(
        aug1.astype(bf).reshape(NG, G, 4, N).transpose(0, 2, 1, 3)
    ).reshape(NG, 4, G * N)
    yq1 = (yq[:, :, None] == np.arange(WAYS)[None, None, :]).astype(np.float32)
    yqp = np.ascontiguousarray(
        yq1.reshape(NB, B, QS, WAYS).transpose(0, 2, 1, 3)
    ).reshape(NB, QS, B * WAYS).astype(bf)
    out = {"x0g": x0g, "xtg": xtg, "augg": augg, "yqp": yqp}
    out.update(host_consts(B))
    return out


def declare_dram(nc, R, NB, B):
    BW = B * WAYS
    NG = R // G
    mk = lambda n, s, dt, k="ExternalInput": nc.dram_tensor(n, s, dt, kind=k).ap()
    return {
        "x0g": mk("x0g", [NG, N, G * DIM], BF),
        "xtg": mk("xtg", [NG, 128, 5 * G * N], BF),
        "augg": mk("augg", [NG, 4, G * N], BF),
        "yqp": mk("yqp", [NB, QS, BW], BF),
        "ident": mk("ident", [128, 128], BF),
        "eyeN": mk("eyeN", [N, N], F32),
        "negEyeN": mk("negEyeN", [N, N], F32),
        "halfI": mk("halfI", [N, N], F32),
        "twoI": mk("twoI", [N, N], F32),
        "twoIbf": mk("twoIbf", [N, N], BF),
        "nid": mk("nid", [N, N], BF),
        "msp": mk("msp", [N, BW], BF),
        "ohp": mk("ohp", [NS, BW], BF),
        "ones_col": mk("ones_col", [128, 1], BF),
        "ones_row": mk("ones_row", [1, 128], BF),
        "ones_colf": mk("ones_colf", [128, 1], F32),
        "ones_rowf": mk("ones_rowf", [1, 128], F32),
        "qmask": mk("qmask", [N, 1], F32),
        "onesw": mk("onesw", [1, G * N], BF),
        "acc": mk("acc", [R], F32, "ExternalOutput"),
    }


def bc3(ap, g):
    """[100,100] const -> broadcast view [100, g, 100] (step-0 middle dim)."""
    return ap.unsqueeze(1).broadcast_to((ap.shape[0], g, ap.shape[1]))


class Kernel:
    def __init__(self, tc, ctx, d, B, debug=()):
        self.tc, self.ctx, self.d, self.B = tc, ctx, d, B
        self.BW = B * WAYS
        self.nc = tc.nc
        self.debug = set(debug)
        self.dbg_tensors = {}
        p = lambda name, bufs, **kw: ctx.enter_context(
            tc.tile_pool(name=name, bufs=bufs, **kw))
        self.consts = p("consts", 1)
        self.xin = p("xin", 3)
        self.gwork = p("gwork", 3)
        self.small = p("small", 4)
        self.store = p("store", 2)
        self.otp = p("otp", 3)
        # PSUM budget (8 banks x 2KB):
        #   gram (m1/m2 [100,500]f32)          bufs=2 -> 2 banks
        #   pf   (op/op2/newton [100,500]f32)  bufs=3 -> 3 banks
        #   bf   (ctp/mtp/rows [<=128,500]bf)  bufs=2 -> 2 banks
        #   p2   (phase2 [100,125]f32)         bufs=1 -> 1 bank
        self.psum_gram = p("psum_gram", 2, space="PSUM")
        self.psum_f = p("psum_f", 2, space="PSUM")
        self.psum_bf = p("psum_bf", 2, space="PSUM")
        self.psum_2 = p("psum_2", 2, space="PSUM")
        self._load_consts()

    def _load_consts(self):
        nc, d = self.nc, self.d
        P = self.consts
        for nm, shape, dt in [
            ("ident", [128, 128], BF), ("eyeN", [N, N], F32),
            ("negEyeN", [N, N], F32),
            ("halfI", [N, N], F32), ("twoI", [N, N], F32),
            ("twoIbf", [N, N], BF),
            ("nid", [N, N], BF),
            ("msp", [N, self.BW], BF),
            ("ones_col", [128, 1], BF), ("ones_row", [1, 128], BF),
            ("ones_colf", [128, 1], F32), ("ones_rowf", [1, 128], F32),
            ("qmask", [N, 1], F32),
            ("onesw", [1, G * N], BF),
        ]:
            t = P.tile(shape, dt, tag=nm)
            nc.sync.dma_start(t[:], d[nm][:])
            setattr(self, nm, t)
        self.rsqmagic = P.tile([128, 8], mybir.dt.uint32, tag="rsqmagic")
        nc.vector.memset(self.rsqmagic[:], 0x5F3759DF)
        self.ohp = P.tile([N, self.BW], BF, tag="ohp")
        nc.vector.memset(self.ohp[0:QS, :], 0.0)
        nc.sync.dma_start(self.ohp[QS:N, :], d["ohp"][:])

    def tap(self, name, ap, r):
        if name not in self.debug:
            return
        nc = self.nc
        key = f"dbg_{name}_{r}"
        t = nc.dram_tensor(key, list(ap.shape), ap.dtype, kind="ExternalOutput").ap()
        self.dbg_tensors[key] = t
        if ap.space == bass.MemorySpace.PSUM:
            s = self.gwork.tile(list(ap.shape), ap.dtype, tag="dbgcpy")
            nc.vector.tensor_copy(s[:], ap)
            ap = s[:]
        nc.sync.dma_start(t, ap)

    def recip(self, out, in_):
        if USE_RECIP_APPROX:
            self.nc.vector.reciprocal_approx_fast(out, in_)
        else:
            self.nc.vector.reciprocal(out, in_)

    # ---------------------------------------------------------------- phase 1
    def degree_rows(self, rs_g, want_scaled):
        """rs_g [100,G] f32 rowsums -> per-run [1,100] bf16 rows of rs^-0.5
        (and alpha*rs^-0.5 if want_scaled). DVE-only fast inverse sqrt
        (bit trick + 2 Newton steps, ~5e-6 rel err) — keeps the ACT table
        set at {exp, copy, relu} for the whole program."""
        nc = self.nc
        U32 = mybir.dt.uint32
        iv = self.small.tile([N, G], U32, tag="rsq_i")
        nc.vector.tensor_scalar(iv[:], rs_g[:].bitcast(U32), 1, None,
                                alu.logical_shift_right)
        nc.vector.tensor_tensor(iv[:], self.rsqmagic[:N, :G], iv[:],
                                alu.subtract)
        yv = self.small.tile([N, G], F32, tag="rsq_y")
        tv = self.small.tile([N, G], F32, tag="rsq_t")
        nc.vector.tensor_copy(yv[:], iv[:].bitcast(F32))
        for _ in range(2):
            nc.vector.tensor_tensor(tv[:], yv[:], yv[:], alu.mult)
            nc.vector.tensor_tensor(tv[:], tv[:], rs_g[:], alu.mult)
            nc.vector.tensor_scalar(tv[:], tv[:], -0.5, 1.5, alu.mult, alu.add)
            nc.vector.tensor_tensor(yv[:], yv[:], tv[:], alu.mult)
        dm_p = self.small.tile([N, G], BF, tag="dm_p")
        nc.vector.tensor_copy(dm_p[:], yv[:])
        dm_s = None
        if want_scaled:
            dm_s = self.small.tile([N, G], BF, tag="dm_s")
            nc.vector.tensor_scalar(dm_s[:], yv[:], ALPHA, None, alu.mult)
        ppb = self.psum_bf.tile([128, G * N], BF, tag="bf")
        for i in range(G):
            nc.tensor.transpose(ppb[0:1, i * N:(i + 1) * N], dm_p[:, i:i + 1],
                                self.ident[:N, :N])
        rpb = self.small.tile([1, G * N], BF, tag="rpb")
        nc.scalar.activation(rpb[:], ppb[0:1, :], actf.Copy)
        rows_p = [rpb[:, i * N:(i + 1) * N] for i in range(G)]
        rows_s = []
        if want_scaled:
            spb = self.psum_bf.tile([128, G * N], BF, tag="bf")
            for i in range(G):
                nc.tensor.transpose(spb[0:1, i * N:(i + 1) * N], dm_s[:, i:i + 1],
                                    self.ident[:N, :N])
            rsb = self.small.tile([1, G * N], BF, tag="rsb")
            nc.scalar.activation(rsb[:], spb[0:1, :], actf.Copy)
            rows_s = [rsb[:, i * N:(i + 1) * N] for i in range(G)]
        return rows_p, rows_s

    def stage1(self, gi):
        """Chunk gi: gram1 -> a1 -> degree -> G^4. Returns (x0, g4)."""
        nc, d = self.nc, self.d
        GW = G * N
        r3 = lambda t: t.rearrange("p (g n) -> p g n", n=N)
        x0 = self.xin.tile([N, G * DIM], BF, tag="x0", bufs=6)
        nc.sync.dma_start(x0[:], d["x0g"][gi])
        xt = self.xin.tile([128, 5 * GW], BF, tag="xt", bufs=4)
        nc.sync.dma_start(xt[:], d["xtg"][gi])
        augl = self.xin.tile([2, GW], BF, tag="augl")
        nc.sync.dma_start(augl[:], d["augg"][gi, 0:2])
        augr = self.xin.tile([2, GW], BF, tag="augr")
        nc.sync.dma_start(augr[:], d["augg"][gi, 2:4])
        xts = lambda k, i: xt[:, (k * G + i) * N:(k * G + i + 1) * N]
        # gram1 (halved): m1' = qn/2_i + qn/2_j - x_i.x_j
        m1 = self.psum_gram.tile([N, GW], F32, tag="gram")
        for i in range(G):
            sl = slice(i * N, (i + 1) * N)
            for k in range(5):
                nc.tensor.matmul(m1[:, sl], xts(k, i), xts(k, i),
                                 start=(k == 0), stop=False)
            nc.tensor.matmul(m1[:, sl], augl[:, sl], augr[:, sl],
                             start=False, stop=True)
        a1 = self.gwork.tile([N, GW], F32, tag="a1")
        nc.scalar.activation(a1[:], m1[:], actf.Exp, scale=2.0 * LAM)
        self.tap("a1", a1[:], gi)
        rs1 = self.small.tile([N, G], F32, tag="rs1")
        nc.vector.tensor_reduce(rs1[:], r3(a1[:]), axl.X, alu.add)
        rows_p, _ = self.degree_rows(rs1, want_scaled=False)
        op = self.psum_f.tile([N, GW], F32, tag="pf")
        for i in range(G):
            nc.tensor.matmul(op[:, i * N:(i + 1) * N], rows_p[i], rows_p[i])
        gw = self.gwork.tile([N, GW], F32, tag="gw")
        nc.vector.tensor_tensor(gw[:], op[:], a1[:], alu.mult)
        gh = self.gwork.tile([N, GW], F32, tag="gh")
        nc.gpsimd.tensor_tensor(r3(gh[:]), r3(gw[:]),
                                bc3(self.eyeN[:], G), alu.add)
        gh2 = self.gwork.tile([N, GW], F32, tag="gh2")
        nc.gpsimd.tensor_tensor(gh2[:], gh[:], gh[:], alu.mult)
        g4 = self.gwork.tile([N, GW], BF, tag="g4", bufs=6)
        nc.gpsimd.tensor_tensor(g4[:], gh2[:], gh2[:], alu.mult)
        # g4 = (2*Gh)^4 = 16*Gh^4; the 1/16 is folded into the ct copy scale
        self.tap("g4", g4[:], gi)
        return x0, g4

    def stage2(self, x0, g4, m2a_sl):
        """Chunk: conv (directly transposed), gram2 + aug + diag penalty,
        m2a store, exp2, top-k -> (w2, rs2).

        XcT = X0^T @ G4 exploits G4's symmetry: no transposes and no
        untransposed Xc at all; qn2 comes from a mid-chain diag read."""
        nc = self.nc
        GW = G * N
        r3 = lambda t: t.rearrange("p (g n) -> p g n", n=N)
        m2 = self.psum_gram.tile([N, GW], F32, tag="gram")
        labq = self.small.tile([1, G * N], BF, tag="labq")
        for i in range(G):
            sl = slice(i * N, (i + 1) * N)
            ctp = self.psum_f.tile([128, 5 * N], F32, tag="pf")
            for k in range(5):
                nc.tensor.matmul(ctp[:, k * N:(k + 1) * N],
                                 x0[:, i * DIM + 128 * k:i * DIM + 128 * (k + 1)],
                                 g4[:, i * N:(i + 1) * N])
            ct = self.xin.tile([128, 5 * N], BF, tag="xct", bufs=6)
            nc.scalar.activation(ct[:], ctp[:], actf.Copy, scale=1.0 / 16.0)
            self.tap("ct", ct[:], self._dbg_c * 10 + i)
            # squared node norms via ones-vector colsum of ct.^2 (per run row)
            sq = self.xin.tile([128, 5 * N], F32, tag="sq", bufs=6)
            nc.vector.tensor_tensor(sq[:], ct[:], ct[:], alu.mult)
            qp = self.psum_f.tile([1, 5 * N], F32, tag="pf")
            for k in range(5):
                nc.tensor.matmul(qp[0:1, 0:N], self.ones_colf[:, :],
                                 sq[:, k * N:(k + 1) * N],
                                 start=(k == 0), stop=(k == 4))
            nc.scalar.activation(labq[0:1, sl], qp[0:1, 0:N], actf.Copy,
                                 scale=-0.5)
            # gram2 for this run, then aug rank-1s + diag penalty; each
            # run's chain closes before the next one starts (PSUM start
            # bits are bank-granular)
            for k in range(5):
                nc.tensor.matmul(m2[:, sl], ct[:, k * N:(k + 1) * N],
                                 ct[:, k * N:(k + 1) * N],
                                 start=(k == 0), stop=False)
            nc.tensor.matmul(m2[:, sl], labq[0:1, sl], self.onesw[0:1, sl],
                             start=False, stop=False)
            nc.tensor.matmul(m2[:, sl], self.onesw[0:1, sl], labq[0:1, sl],
                             start=False, stop=False)
            nc.tensor.matmul(m2[:, sl], self.eyeN_bf_lhsT(), self.nid[:],
                             start=False, stop=True)
        nc.scalar.activation(m2a_sl, m2[:], actf.Copy)     # bf16 store
        self.tap("m2a", m2a_sl, self._dbg_c)
        exp2 = self.gwork.tile([N, GW], F32, tag="exp2")
        nc.scalar.activation(exp2[:], m2[:], actf.Exp, scale=2.0 * LAM)
        # top-k threshold + masked W rows
        w2 = self.gwork.tile([N, GW], F32, tag="w2", bufs=6)
        rs2 = self.small.tile([N, G], F32, tag="rs2", bufs=6)
        for i in range(G):
            sl = slice(i * N, (i + 1) * N)
            m8 = self.small.tile([N, 8], F32, tag="m8")
            cp2 = self.gwork.tile([N, N], F32, tag="cp2")
            cp3 = self.gwork.tile([N, N], F32, tag="cp3")
            nc.vector.max(m8[:], exp2[:, sl])
            nc.vector.match_replace(cp2[:], m8[:], exp2[:, sl], -1.0)
            nc.vector.max(m8[:], cp2[:])
            nc.vector.match_replace(cp3[:], m8[:], cp2[:], -1.0)
            m8c = self.small.tile([N, 8], F32, tag="m8c")
            nc.vector.max(m8c[:], cp3[:])
            nc.vector.scalar_tensor_tensor(w2[:, sl], exp2[:, sl], m8c[:, 3:4],
                                           exp2[:, sl], alu.is_ge, alu.mult,
                                           accum_out=rs2[:, i:i + 1])
        return w2, rs2

    def stage3(self, w2, rs2, y_sl):
        """Chunk: symmetric normalize -> alpha*W -> Newton inverse -> y_sl."""
        nc = self.nc
        GW = G * N
        r3 = lambda t: t.rearrange("p (g n) -> p g n", n=N)
        self.tap("w2", w2[:], self._dbg_c)
        self.tap("rs2", rs2[:], self._dbg_c)
        rows_p2, rows_s2 = self.degree_rows(rs2, want_scaled=True)
        op2 = self.psum_f.tile([N, GW], F32, tag="pf")
        for i in range(G):
            nc.tensor.matmul(op2[:, i * N:(i + 1) * N], rows_p2[i], rows_s2[i])
        mmb = self.gwork.tile([N, GW], BF, tag="mmb")
        nc.vector.tensor_tensor(mmb[:], op2[:], w2[:], alu.mult)   # alpha*W
        self.tap("mm", mmb[:], self._dbg_c)
        # Newton; bn = M - I so the 2I-folds ride the PE accumulation
        mtp = self.psum_bf.tile([128, GW], BF, tag="bf")
        for i in range(G):
            nc.tensor.transpose(mtp[0:N, i * N:(i + 1) * N],
                                mmb[:, i * N:(i + 1) * N], self.ident[:N, :N])
        y = self.gwork.tile([N, GW], BF, tag="y")
        nc.vector.scalar_tensor_tensor(r3(y[:]), r3(mtp[0:N, :]), 1.0,
                                       bc3(self.eyeN[:], G), alu.mult, alu.add)
        bn = self.gwork.tile([N, GW], BF, tag="bn")
        nc.gpsimd.tensor_tensor(r3(bn[:]), r3(mmb[:]),
                                bc3(self.eyeN[:], G), alu.subtract)
        yt = self.gwork.tile([N, GW], BF, tag="yt")
        nc.gpsimd.tensor_tensor(r3(yt[:]), r3(mmb[:]),
                                bc3(self.eyeN[:], G), alu.add)
        for it in range(NEWTON_ITERS):
            last = it == NEWTON_ITERS - 1
            # u = 2I - B@Y = bn@y + 2I, with the 2I added by the PE chain
            tp = self.psum_f.tile([N, GW], F32, tag="pf")
            for i in range(G):
                sl = slice(i * N, (i + 1) * N)
                nc.tensor.matmul(tp[:, sl], bn[:, sl], y[:, sl],
                                 start=True, stop=False)
                nc.tensor.matmul(tp[:, sl], self.ident[:N, :N], self.twoIbf[:],
                                 start=False, stop=True)
            u = self.gwork.tile([N, GW], BF, tag="u")
            nc.vector.tensor_copy(u[:], tp[:])
            ynp = self.psum_f.tile([N, GW], F32, tag="pf")
            for i in range(G):
                sl = slice(i * N, (i + 1) * N)
                nc.tensor.matmul(ynp[:, sl], yt[:, sl], u[:, sl])
            if last:
                nc.scalar.activation(y_sl, ynp[:], actf.Copy)
            else:
                yn = self.gwork.tile([N, GW], BF, tag="y")
                nc.scalar.activation(yn[:], ynp[:], actf.Copy)
                sp = self.psum_f.tile([N, GW], F32, tag="pf")
                for i in range(G):
                    sl = slice(i * N, (i + 1) * N)
                    nc.tensor.matmul(sp[:, sl], y[:, sl], bn[:, sl])
                ut = self.gwork.tile([N, GW], BF, tag="u2")
                nc.vector.scalar_tensor_tensor(r3(ut[:]), r3(sp[:]), 1.0,
                                               bc3(self.twoI[:], G), alu.mult,
                                               alu.add)
                ytp = self.psum_f.tile([N, GW], F32, tag="pf")
                for i in range(G):
                    sl = slice(i * N, (i + 1) * N)
                    nc.tensor.matmul(ytp[:, sl], ut[:, sl], yt[:, sl])
                ytn = self.gwork.tile([N, GW], BF, tag="yt")
                nc.scalar.activation(ytn[:], ytp[:], actf.Copy)
                y, yt = yn, ytn
        self.tap("y", y_sl, self._dbg_c)

    _eyeN_bf = None

    def eyeN_bf_lhsT(self):
        if self._eyeN_bf is None:
            self._eyeN_bf = self.ident[:N, :N]
        return self._eyeN_bf

    # ---------------------------------------------------------------- phase 2
    def colsum_mm(self, src_ap, n_rows):
        nc = self.nc
        ones = self.ones_colf if src_ap.dtype == F32 else self.ones_col
        cs = self.psum_2.tile([1, self.BW], F32, tag="p2")
        nc.tensor.matmul(cs[:], ones[:n_rows, :], src_ap)
        return cs

    def bcast_mm(self, row_ap):
        nc = self.nc
        bc = self.psum_2.tile([N, self.BW], F32, tag="p2")
        ones = self.ones_rowf if row_ap.dtype == F32 else self.ones_row
        nc.tensor.matmul(bc[:], ones[:, :N], row_ap)
        return bc

    def sinkhorn(self, P, n_rows, c_val, clamp):
        nc, B, BW = self.nc, self.B, self.BW
        for _ in range(SINK_ITERS):
            u = self.otp.tile([n_rows, B], F32, tag="u")
            p3 = P[0:n_rows, :].rearrange("p (r w) -> p r w", w=WAYS)
            nc.vector.tensor_reduce(u[:], p3, axl.X, alu.add)
            ui = self.otp.tile([n_rows, B], F32, tag="ui")
            self.recip(ui[:], u[:])
            uib = ui[:].unsqueeze(2).broadcast_to((n_rows, B, WAYS))
            nc.gpsimd.tensor_tensor(p3, p3, uib, alu.mult)
            cs = self.colsum_mm(P[0:n_rows, :], n_rows)
            cf = self.otp.tile([1, BW], F32, tag="cf")
            self.recip(cf[:], cs[:])
            bc = self.bcast_mm(cf[:])
            nc.vector.scalar_tensor_tensor(P[0:n_rows, :], bc[0:n_rows, :],
                                           c_val, P[0:n_rows, :],
                                           alu.mult, alu.mult)
            if clamp:
                nc.vector.scalar_tensor_tensor(P[:], P[:], self.qmask[:],
                                               self.ohp[:], alu.mult, alu.add)

    def dist_exp(self, zt, m2a_store, P, t5pack):
        nc, B, BW = self.nc, self.B, self.BW
        t5p = self.psum_2.tile([N, BW], F32, tag="p2")
        for i in range(B):
            nc.tensor.matmul(t5p[:, i * WAYS:(i + 1) * WAYS],
                             m2a_store[:, i * N:(i + 1) * N],
                             zt[:, i * WAYS:(i + 1) * WAYS])
        h = self.otp.tile([N, BW], F32, tag="h")
        nc.vector.tensor_tensor(h[:], t5p[:], zt[:], alu.mult)
        zmz = self.colsum_mm(h[:], N)
        epn = self.otp.tile([1, BW], F32, tag="epn")
        nc.scalar.activation(epn[:], zmz[:], actf.Exp, scale=-LAM)
        nc.scalar.activation(P[0:QS, :], t5p[0:QS, :], actf.Exp,
                             scale=2.0 * LAM)
        bc = self.bcast_mm(epn[:])
        nc.vector.tensor_tensor(P[0:QS, :], P[0:QS, :], bc[0:QS, :], alu.mult)

    def phase2_batch(self, b, m2a_store, y_store):
        nc, d, B, BW = self.nc, self.d, self.B, self.BW
        yq = self.otp.tile([QS, BW], BF, tag="yq")
        nc.sync.dma_start(yq[:], d["yqp"][b])
        P = self.otp.tile([N, BW], BF, tag="P")
        nc.vector.tensor_copy(P[:], self.ohp[:])
        zt = self.otp.tile([N, BW], BF, tag="zt")
        nc.vector.tensor_copy(zt[:], self.msp[:])
        t5pack = None
        for ep in range(EPOCHS):
            self.dist_exp(zt, m2a_store, P, t5pack)
            self.sinkhorn(P, QS, float(QS // WAYS), clamp=False)
            zap = self.psum_2.tile([N, BW], F32, tag="p2")
            for i in range(B):
                nc.tensor.matmul(zap[:, i * WAYS:(i + 1) * WAYS],
                                 y_store[:, i * N:(i + 1) * N],
                                 P[:, i * WAYS:(i + 1) * WAYS])
            nc.scalar.activation(P[:], zap[:], actf.Relu)
            self.sinkhorn(P, N, float(N // WAYS), clamp=True)
            csz = self.colsum_mm(P[:], N)
            ci = self.otp.tile([1, BW], F32, tag="cf")
            self.recip(ci[:], csz[:])
            bcz = self.bcast_mm(ci[:])
            t = self.otp.tile([N, BW], F32, tag="h")
            nc.vector.scalar_tensor_tensor(t[:], bcz[:], UR, P[:],
                                           alu.mult, alu.mult)
            ztn = self.otp.tile([N, BW], BF, tag="zt")
            nc.vector.scalar_tensor_tensor(ztn[:], zt[:], 1.0 - UR, t[:],
                                           alu.mult, alu.add)
            zt = ztn
        self.dist_exp(zt, m2a_store, P, t5pack)
        self.sinkhorn(P, QS, float(QS // WAYS), clamp=False)
        if "pfin" in self.debug:
            self.tap("pfin", P[:], b)
        pt = self.otp.tile([QS, BW], F32, tag="pt")
        nc.gpsimd.tensor_tensor(pt[:], P[0:QS, :], yq[:], alu.mult)
        ptr = self.otp.tile([QS, B], F32, tag="ptr")
        nc.vector.tensor_reduce(ptr[:], pt[:].rearrange("p (r w) -> p r w", w=WAYS),
                                axl.X, alu.add)
        pmx = self.otp.tile([QS, B], F32, tag="pmx")
        nc.vector.tensor_reduce(pmx[:], P[0:QS, :].rearrange("p (r w) -> p r w", w=WAYS),
                                axl.X, alu.max)
        ok = self.otp.tile([QS, B], BF, tag="ok")
        nc.vector.tensor_tensor(ok[:], ptr[:], pmx[:], alu.is_ge)
        am = self.psum_2.tile([1, B], F32, tag="p2")
        nc.tensor.matmul(am[:], self.ones_col[:QS, :], ok[:])
        accs = self.otp.tile([1, B], F32, tag="accs")
        nc.scalar.activation(accs[:], am[:], actf.Copy, scale=1.0 / QS)
        nc.sync.dma_start(d["acc"][b * B:(b + 1) * B].unsqueeze(0), accs[:])

    def run_all(self, R, NB, repeat=1):
        def body():
            B = self.B
            NGB = B // G       # chunks per batch
            for b in range(NB):
                m2a_store = self.store.tile([N, B * N], BF, tag="m2a_store")
                y_store = self.store.tile([N, B * N], BF, tag="y_store")
                # stage-major emission: all chunks through each stage so the
                # scheduler always has independent per-chunk work in flight
                s1 = [self.stage1(b * NGB + c) for c in range(NGB)]
                s2 = []
                for c in range(NGB):
                    self._dbg_c = b * NGB + c
                    s2.append(self.stage2(s1[c][0], s1[c][1],
                                          m2a_store[:, c * G * N:(c + 1) * G * N]))
                for c in range(NGB):
                    self._dbg_c = b * NGB + c
                    self.stage3(s2[c][0], s2[c][1],
                                y_store[:, c * G * N:(c + 1) * G * N])
                self.phase2_batch(b, m2a_store, y_store)
        if repeat == 1:
            body()
        elif repeat < 0:
            # inline-unrolled variant (no loop machinery), for measuring
            # steady-state body time without For_i drain overhead
            for _ in range(-repeat):
                body()
        else:
            # measurement aid: execute the whole workload `repeat` times on
            # device so (t[repeat=K] - t[repeat=1])/(K-1) isolates pure HW
            # execution time from host/tunnel dispatch latency.
            with self.tc.For_i(0, repeat):
                body()


def build(R, B, num_devices=8, debug=(), trn="TRN2", repeat=1):
    NB = R // B
    assert NB * B == R and B % G == 0
    nc = bacc.Bacc(trn, target_bir_lowering=False, debug=False,
                   enable_asserts=True, num_devices=num_devices)
    d = declare_dram(nc, R, NB, B)
    with tile.TileContext(nc) as tc:
        with ExitStack() as ctx:
            k = Kernel(tc, ctx, d, B, debug=debug)
            k.run_all(R, NB, repeat=repeat)
    nc.compile()
    return nc, d, k.dbg_tensors


# ----------------------------------------------------------------- entry point
_CACHE = {}

N_CORES = 8
R_TOTAL = 1000
R_CORE = R_TOTAL // N_CORES      # 125
BATCH = 25


def kernel(xs, xq, ys, yq):
    """Full inputs in, full output out. xs [1000,25,640] f32, xq [1000,75,640]
    f32, ys [1000,25] i32, yq [1000,75] i32 -> acc [1000] f32."""
    from concourse import bass_utils

    xs = np.asarray(xs, dtype=np.float32)
    xq = np.asarray(xq, dtype=np.float32)
    yq = np.asarray(yq, dtype=np.int32)

    if "nc" not in _CACHE:
        _CACHE["nc"] = build(R_CORE, BATCH, num_devices=N_CORES)[0]
    nc = _CACHE["nc"]

    in_maps = []
    for c in range(N_CORES):
        sl = slice(c * R_CORE, (c + 1) * R_CORE)
        in_maps.append(host_inputs(xs[sl], xq[sl], yq[sl],
                                   R_CORE // BATCH, BATCH))
    res = bass_utils.run_bass_kernel_spmd(nc, in_maps,
                                          core_ids=list(range(N_CORES)))
    return np.concatenate([res.results[c]["acc"] for c in range(N_CORES)])

